# revision 12
# baseline (speedup 1.0000x reference)
"""MultiHeadDiffAttention TRN2 kernel.

Sharding: 8 cores = 2 batches x 4 head-pairs. Core c handles batch c//4 and
heads {2g, 2g+1} where g = c%4. The 2 heads = 128 channels = exactly one
GroupNorm group, so GroupNorm is core-local. The final projection is computed
as a partial sum over the core's 128 channels; the host adds the 4 partials
per batch plus the output bias.

Layout: "channel-major" [channels(partitions), sequence(free)] everywhere.
  - host pre-transposes/packs x and weights into per-partition-contiguous
    layouts so each big input is a single DMA
  - q/k projections land as qT/kT [128(2 heads x 64hd), 2048]
  - scores are computed transposed: S_T[k, q] (keys on partitions) so the
    exp'd scores feed the attn@V matmul directly as the moving operand
  - v is produced token-major [s, hd] with a ones-column appended, so the
    attn@V matmul also yields the softmax denominator (row 64 of PSUM out)
  - softmax skips max-subtraction: scores are bounded (|s|<2 for this data
    distribution, exp is exact in fp32)
  - per-(head, attn) exp-weight sums U are staged to SBUF; the diff-attn
    combine runs once per head over the full [64, 2048] row

The execution environment charges a large fixed cost per instruction, so the
structure minimizes total instruction count: exp in [128, 2048] groups,
single packed DMAs, full-row combine, one packed output store.
"""

import os
import sys

sys.path.insert(0, "/opt/trn_rl_repo")

import numpy as np

import concourse.bacc as bacc
import concourse.bass as bass
import concourse.mybir as mybir
import concourse.tile as tile
from concourse.masks import make_identity
from concourse.bass_utils import run_bass_kernel_spmd

B, S, D = 2, 2048, 512
H = 8
G = 4
HD = D // H          # 64
CH = 2 * HD          # 128 channels per core (one GroupNorm group)
LAMBDA_INIT = 0.2
EPS = 1e-5
N_CORES = 8

QB = 512             # query block (matmul N)
NQB = S // QB        # 4
KB = 128             # key block (matmul M)
NKB = S // KB        # 16
KG = 2               # key blocks per exp group ([128, 1024] PSUM tile);
                     # small enough that sc_pool can double-buffer (2 banks
                     # x 2 bufs) next to the 4-bank av accumulator, so the
                     # PE never stalls waiting for the exp to drain PSUM
NKG = NKB // KG      # 8
SB = 128             # seq block for v / final matmul
NSB = S // SB        # 16

F32 = mybir.dt.float32
F32R = mybir.dt.float32r
BF16 = mybir.dt.bfloat16

USE_BF16 = os.environ.get("KERNEL_BF16", "0") == "1"
MMDT = BF16 if USE_BF16 else F32R
NWEIGHTS = 5

_CACHE = {}


def r(ap):
    """bitcast an fp32-typed AP to float32r (no-op for bf16 tiles)"""
    if USE_BF16:
        return ap
    return ap.bitcast(F32R)


def build_program(repeats=1):
    nc = bacc.Bacc("TRN2", target_bir_lowering=False, debug=False)

    mmdt_in = MMDT if USE_BF16 else F32

    # ---- external I/O (packed per-partition-contiguous host layouts) ----
    # xp[p, c, s] = x[b, s, 128c+p]
    d_xp = nc.declare_dram_parameter("xp", [128, 4 * S], mmdt_in, isOutput=False)
    # wp[p, w, c, m] = W_w[ch0+m, 128c+p]; w in (q1,k1,q2,k2,v); + owT at tail
    d_wp = nc.declare_dram_parameter("wp", [128, NWEIGHTS * 512 + D],
                                     mmdt_in, isOutput=False)
    # cp[p, :] = [k1b, k2b, gnw, gnb, neglam0, neglam1, neglam_packed]
    d_cp = nc.declare_dram_parameter("cp", [CH, 7], F32, isOutput=False)
    # yp[p, sb, d] = y_part[128*sb+p, d]
    d_y = nc.declare_dram_parameter("y_part", [SB, NSB * D], F32, isOutput=True)

    with tile.TileContext(nc) as tc:
      for _rep in range(repeats):
        with (
            tc.tile_pool(name="consts", bufs=1) as consts,
            tc.tile_pool(name="qk", bufs=1) as qk_pool,
            tc.tile_pool(name="vaug", bufs=1) as vaug_pool,
        ):
            # ---- constants / packed inputs ----
            ones = consts.tile([128, 1], F32, tag="ones")
            nc.vector.memset(ones, 1.0)
            onesr = consts.tile([1, 128], F32, tag="onesr")
            nc.vector.memset(onesr, 1.0)
            eps_t = consts.tile([1, 1], F32, tag="eps")
            nc.vector.memset(eps_t, EPS)
            cp = consts.tile([CH, 7], F32, tag="cp")
            nc.sync.dma_start(out=cp, in_=d_cp.ap())
            k1b, k2b = cp[:, 0:1], cp[:, 1:2]
            gnw, gnb = cp[:, 2:3], cp[:, 3:4]
            neglam = cp[:, 4:6]
            neglamc = cp[:, 6:7]

            wt = consts.tile([128, NWEIGHTS, 4, CH], MMDT, tag="wt")
            owT = consts.tile([CH, D], MMDT, tag="owT")
            wp_ap = d_wp.ap() if USE_BF16 else d_wp.ap().bitcast(F32R)
            nc.sync.dma_start(
                out=wt,
                in_=wp_ap[:, 0:NWEIGHTS * 512].rearrange(
                    "p (w c m) -> p w c m", w=NWEIGHTS, c=4))
            nc.sync.dma_start(out=owT, in_=wp_ap[:, NWEIGHTS * 512:])
            WIDX = {"q1": 0, "k1": 1, "q2": 2, "k2": 3, "v": 4}

            # ---- projections: qT/kT [128, 2048] channel-major ----
            qk = {}
            with (
                tc.tile_pool(name="xtp", bufs=1) as xt_pool,
                tc.tile_pool(name="pj", bufs=1, space="PSUM") as pj_pool,
                tc.tile_pool(name="pv", bufs=4, space="PSUM") as pv_pool,
            ):
                xt = xt_pool.tile([128, 4, S], MMDT, tag="xt")
                nc.sync.dma_start(
                    out=xt, in_=d_xp.ap().rearrange("p (c s) -> p c s", c=4)
                    if USE_BF16 else
                    d_xp.ap().bitcast(F32R).rearrange("p (c s) -> p c s", c=4))

                for w, bias in (("k1", k1b), ("q1", None), ("k2", k2b),
                                ("q2", None)):
                    dst = qk_pool.tile([CH, S], MMDT, tag=w)
                    qk[w] = dst
                    ps = pj_pool.tile([CH, 4 * QB], F32, tag="pj", name="pj")
                    for qb in range(NQB):
                        for c in range(4):
                            nc.tensor.matmul(
                                ps[:, qb * QB:(qb + 1) * QB],
                                wt[:, WIDX[w], c, :],
                                xt[:, c, qb * QB:(qb + 1) * QB],
                                start=(c == 0),
                                stop=(c == 3),
                            )
                    if bias is not None:
                        nc.vector.tensor_scalar_add(dst, ps, bias)
                    else:
                        nc.vector.tensor_copy(dst, ps)

                # ---- v: projected channel-major like q/k (4 wide MMs),
                # then PE-transposed per 128-block into token-major va
                # [s, hd] with a ones column; slot 2*sb+h ----
                identf = consts.tile([SB, SB], F32, tag="identf")
                make_identity(nc, identf)
                ident = consts.tile([SB, SB], MMDT, tag="ident")
                nc.vector.tensor_copy(ident, identf)
                vT = qk_pool.tile([CH, S], MMDT, tag="vT")
                ps = pj_pool.tile([CH, 4 * QB], F32, tag="pj", name="pjv")
                for qb in range(NQB):
                    for c in range(4):
                        nc.tensor.matmul(
                            ps[:, qb * QB:(qb + 1) * QB],
                            wt[:, WIDX["v"], c, :],
                            xt[:, c, qb * QB:(qb + 1) * QB],
                            start=(c == 0),
                            stop=(c == 3),
                        )
                nc.vector.tensor_copy(vT, ps)
                va = vaug_pool.tile([SB, 2 * NSB, HD + 1], MMDT, tag="va")
                nc.vector.tensor_copy(
                    va[:, :, HD:HD + 1],
                    ones.to_broadcast((SB, 2 * NSB, 1)))
                for sb in range(NSB):
                    pst = pv_pool.tile([SB, SB], MMDT, tag="pv", name="pv")
                    nc.tensor.transpose(
                        pst, vT[:, sb * SB:(sb + 1) * SB], ident)
                    nc.vector.tensor_copy(
                        va[:, 2 * sb:2 * sb + 2, 0:HD],
                        pst.rearrange("p (h m) -> p h m", h=2))

            # ---- attention-phase pools (reuse the x-tile region) ----
            with (
                tc.tile_pool(name="upool", bufs=4) as u_pool,
                tc.tile_pool(name="ubig", bufs=1) as ubig_pool,
                tc.tile_pool(name="opool", bufs=1) as o_pool,
                tc.tile_pool(name="small", bufs=1) as small,
            ):
              # ---- attention ----
              oT = o_pool.tile([CH, S], F32, tag="oT")
              # U[(h, attn)]: [65, 2048] exp-weight sums staged from PSUM;
              # rr[(h, attn)]: per-token reciprocal denominators
              U = {(h, a): ubig_pool.tile([HD + 1, S], F32, tag=f"U{h}{a}",
                                          name=f"U{h}{a}")
                   for h in (0, 1) for a in (1, 2)}
              rr = {(h, a): small.tile([1, S], F32, tag=f"rr{h}{a}",
                                       name=f"rr{h}{a}")
                    for h in (0, 1) for a in (1, 2)}

              with (
                  tc.tile_pool(name="sc", bufs=2, space="PSUM") as sc_pool,
                  tc.tile_pool(name="av", bufs=1, space="PSUM") as av_pool,
              ):
                  for attn in (1, 2):
                      qT, kT = qk[f"q{attn}"], qk[f"k{attn}"]
                      for h in (0, 1):
                          hs = slice(h * HD, (h + 1) * HD)
                          av = av_pool.tile([HD + 1, NQB * QB], F32,
                                            tag="av", name="av")
                          for qb in range(NQB):
                              for kg in range(NKG):
                                  sct = sc_pool.tile([128, KG * QB], F32,
                                                     tag="sc", name="sc")
                                  for j in range(KG):
                                      kb = kg * KG + j
                                      nc.tensor.matmul(
                                          sct[:, j * QB:(j + 1) * QB],
                                          r(kT[hs, kb * KB:(kb + 1) * KB]),
                                          r(qT[hs, qb * QB:(qb + 1) * QB]),
                                          start=True, stop=True,
                                      )
                                  ut = u_pool.tile([128, KG * QB], MMDT, tag="u")
                                  nc.scalar.activation(
                                      out=ut, in_=sct,
                                      func=mybir.ActivationFunctionType.Exp,
                                      scale=1.0 / (HD ** 0.5),
                                  )
                                  for j in range(KG):
                                      kb = kg * KG + j
                                      nc.tensor.matmul(
                                          av[:, qb * QB:(qb + 1) * QB],
                                          r(va[:, 2 * kb + h, :]),
                                          r(ut[:, j * QB:(j + 1) * QB]),
                                          start=(kb == 0),
                                          stop=(kb == NKB - 1),
                                      )
                          # stage U (incl. denominator row) to SBUF, then
                          # the denominator reciprocal from SBUF — both on
                          # DVE, overlapping the next block's matmuls
                          nc.vector.tensor_copy(U[(h, attn)], av)
                          nc.vector.reciprocal(out=rr[(h, attn)],
                                               in_=U[(h, attn)][HD:HD + 1, :])

              # ---- combine: o = U1*rb1 - lam * U2*rb2 ----
              # rb_a[h*64:(h+1)*64, :] = rr[(h, a)] broadcast via K=1 PE
              # matmuls into PSUM (the PE is idle here; avoids gpsimd)
              with tc.tile_pool(name="rbp", bufs=1, space="PSUM") as rb_pool:
                  rb = {a: rb_pool.tile([CH, S], F32, tag=f"rb{a}",
                                        name=f"rb{a}")
                        for a in (1, 2)}
                  for a in (1, 2):
                      for h in (0, 1):
                          for j in range(NQB):
                              nc.tensor.matmul(
                                  rb[a][h * HD:(h + 1) * HD,
                                        j * QB:(j + 1) * QB],
                                  onesr[:, 0:HD],
                                  rr[(h, a)][:, j * QB:(j + 1) * QB],
                                  start=True, stop=True,
                                  skip_group_check=True)
                  for h in (0, 1):
                      hs = slice(h * HD, (h + 1) * HD)
                      t1 = small.tile([HD, S], F32, tag="t1", name=f"t1{h}")
                      nc.vector.tensor_mul(t1, U[(h, 1)][0:HD, :], rb[1][hs])
                      t2 = small.tile([HD, S], F32, tag="t2", name=f"t2{h}")
                      nc.vector.scalar_tensor_tensor(
                          out=t2, in0=U[(h, 2)][0:HD, :],
                          scalar=neglam[0:HD, h:h + 1], in1=rb[2][hs],
                          op0=mybir.AluOpType.mult,
                          op1=mybir.AluOpType.mult,
                      )
                      nc.vector.tensor_add(oT[hs, :], t1, t2)

              # ---- GroupNorm (whole [128, 2048] is one group) ----
              xn = o_pool.tile([CH, S], MMDT, tag="xn")
              with tc.tile_pool(name="stp", bufs=1, space="PSUM") as stp_pool:
                  nst = S // nc.vector.BN_STATS_FMAX
                  bstats = small.tile([CH, nst, nc.vector.BN_STATS_DIM], F32,
                                      tag="bstats")
                  for i in range(nst):
                      nc.vector.bn_stats(
                          out=bstats[:, i, :],
                          in_=oT[:, i * nc.vector.BN_STATS_FMAX:
                                 (i + 1) * nc.vector.BN_STATS_FMAX])
                  mv = small.tile([CH, nc.vector.BN_AGGR_DIM], F32, tag="mv")
                  nc.vector.bn_aggr(out=mv, in_=bstats)
                  # per-partition [mean, E[x^2]] -> partition-sum via matmul
                  e2c = small.tile([CH, 1], F32, tag="e2c")
                  nc.vector.tensor_mul(e2c, mv[:, 0:1], mv[:, 0:1])
                  nc.vector.tensor_add(e2c, e2c, mv[:, 1:2])
                  st = stp_pool.tile([1, 2], F32, tag="st")
                  nc.tensor.matmul(st[0:1, 0:1], mv[:, 0:1], ones,
                                   start=True, stop=True)
                  nc.tensor.matmul(st[0:1, 1:2], e2c, ones,
                                   start=True, stop=True, skip_group_check=True)
                  mu_e2 = small.tile([1, 2], F32, tag="mu_e2")
                  nc.vector.tensor_scalar_mul(mu_e2, st[0:1, 0:2], 1.0 / CH)
                  # negvar = mu^2 - e2; std = sqrt(-negvar + eps)
                  negvar = small.tile([1, 1], F32, tag="negvar")
                  nc.vector.scalar_tensor_tensor(
                      out=negvar, in0=mu_e2[:, 0:1], scalar=mu_e2[:, 0:1],
                      in1=mu_e2[:, 1:2], op0=mybir.AluOpType.mult,
                      op1=mybir.AluOpType.subtract)
                  std = small.tile([1, 1], F32, tag="std")
                  nc.scalar.activation(out=std, in_=negvar,
                                       func=mybir.ActivationFunctionType.Sqrt,
                                       bias=eps_t, scale=-1.0)
                  rstd = small.tile([1, 1], F32, tag="rstd")
                  nc.vector.reciprocal(out=rstd, in_=std)
                  murstd = small.tile([1, 2], F32, tag="murstd")
                  nc.vector.tensor_copy(murstd[:, 0:1], mu_e2[:, 0:1])
                  nc.vector.tensor_copy(murstd[:, 1:2], rstd)
                  # broadcast [mu, rstd] across partitions via K=1 PE matmul
                  brp = stp_pool.tile([CH, 2], F32, tag="brp")
                  nc.tensor.matmul(brp, onesr, murstd, start=True, stop=True,
                                   skip_group_check=True)
                  a_t = small.tile([CH, 1], F32, tag="a_t")
                  nc.vector.tensor_mul(a_t, brp[:, 1:2], gnw)
                  # negb = a*mu - gnb;  xn = a*oT - negb
                  negb = small.tile([CH, 1], F32, tag="negb")
                  nc.vector.scalar_tensor_tensor(
                      out=negb, in0=brp[:, 0:1], scalar=a_t, in1=gnb,
                      op0=mybir.AluOpType.mult,
                      op1=mybir.AluOpType.subtract)
                  nc.vector.tensor_scalar(out=xn, in0=oT, scalar1=a_t,
                                          scalar2=negb,
                                          op0=mybir.AluOpType.mult,
                                          op1=mybir.AluOpType.subtract)

              # ---- final projection partial: y = xn.T @ owT ----
              # 4 MMs fill a [128, 2048] (4-bank) PSUM tile -> 1 copy each
              with tc.tile_pool(name="fin", bufs=2, space="PSUM") as fin_pool:
                  half = NSB // 4
                  for hf in range(4):
                      ps = fin_pool.tile([SB, half * D], F32, tag="fin",
                                         name="fin")
                      yt = o_pool.tile([SB, half, D], F32, tag="yt", name="yt")
                      for i in range(half):
                          sb = hf * half + i
                          nc.tensor.matmul(
                              ps[:, i * D:(i + 1) * D],
                              r(xn[:, sb * SB:(sb + 1) * SB]),
                              r(owT),
                              start=True, stop=True,
                          )
                      nc.vector.tensor_copy(yt, ps.rearrange(
                          "p (i d) -> p i d", i=half))
                      nc.sync.dma_start(
                          out=d_y.ap().rearrange(
                              "p (hf sb d) -> p hf sb d", hf=4, sb=half)[:, hf],
                          in_=yt)

    nc.compile()
    return nc


def _shard_inputs(inputs):
    import ml_dtypes
    mmnp = ml_dtypes.bfloat16 if USE_BF16 else np.float32
    x = np.ascontiguousarray(inputs["x"], np.float32)
    lam = (np.exp(inputs["lambda_q1"] * inputs["lambda_k1"])
           - np.exp(inputs["lambda_q2"] * inputs["lambda_k2"])
           + LAMBDA_INIT).astype(np.float32).reshape(H)
    in_maps = []
    for c in range(N_CORES):
        b, g = divmod(c, 4)
        ch = slice(CH * g, CH * (g + 1))
        # xp[p, c, s] = x[b, s, 128c+p]
        xp = np.ascontiguousarray(
            x[b].T.reshape(4, 128, S).transpose(1, 0, 2).reshape(128, 4 * S)
        ).astype(mmnp)
        wlist = []
        for W in (inputs["Q1_w"], inputs["K1_w"], inputs["Q2_w"],
                  inputs["K2_w"], inputs["V_w"]):
            wT = np.asarray(W)[ch].T  # [512, 128]
            wlist.append(np.ascontiguousarray(
                wT.reshape(4, 128, CH).transpose(1, 0, 2).reshape(128, 512)))
        owT = np.ascontiguousarray(np.asarray(inputs["out_w"])[:, ch].T)
        wp = np.concatenate(wlist + [owT], axis=1).astype(mmnp)
        neglam_packed = np.concatenate([
            np.full(HD, -lam[2 * g], np.float32),
            np.full(HD, -lam[2 * g + 1], np.float32)])
        cp = np.stack([
            np.asarray(inputs["K1_b"])[ch],
            np.asarray(inputs["K2_b"])[ch],
            np.asarray(inputs["gn_w"])[ch],
            np.asarray(inputs["gn_b"])[ch],
            np.full(CH, -lam[2 * g], np.float32),
            np.full(CH, -lam[2 * g + 1], np.float32),
            neglam_packed,
        ], axis=1).astype(np.float32)
        in_maps.append({"xp": xp, "wp": wp, "cp": np.ascontiguousarray(cp)})
    return in_maps


def kernel(**inputs):
    inputs = {k: np.asarray(v) for k, v in inputs.items()}
    if "nc" not in _CACHE:
        _CACHE["nc"] = build_program()
    nc = _CACHE["nc"]
    in_maps = _shard_inputs(inputs)
    res = run_bass_kernel_spmd(nc, in_maps, list(range(N_CORES)))
    out_b = np.asarray(inputs["out_b"], np.float32)
    y = np.zeros((B, S, D), np.float32)
    for c in range(N_CORES):
        b = c // 4
        yp = res.results[c]["y_part"].astype(np.float32)
        y[b] += yp.reshape(SB, NSB, D).transpose(1, 0, 2).reshape(S, D)
    y += out_b[None, None, :]
    return y



# revision 15
# speedup vs baseline: 1.2235x; 1.2235x over previous
"""MultiHeadDiffAttention TRN2 kernel.

Sharding: 8 cores = 2 batches x 4 head-pairs. Core c handles batch c//4 and
heads {2g, 2g+1} where g = c%4. The 2 heads = 128 channels = exactly one
GroupNorm group, so GroupNorm is core-local. The final projection is computed
as a partial sum over the core's 128 channels; the host adds the 4 partials
per batch plus the output bias.

Layout: "channel-major" [channels(partitions), sequence(free)] everywhere.
  - host pre-transposes/packs x and weights into per-partition-contiguous
    layouts so each big input is a single DMA
  - q/k projections land as qT/kT [128(2 heads x 64hd), 2048]
  - scores are computed transposed: S_T[k, q] (keys on partitions) so the
    exp'd scores feed the attn@V matmul directly as the moving operand
  - v is produced token-major [s, hd] with a ones-column appended, so the
    attn@V matmul also yields the softmax denominator (row 64 of PSUM out)
  - softmax skips max-subtraction: scores are bounded (|s|<2 for this data
    distribution, exp is exact in fp32)
  - per-(head, attn) exp-weight sums U are staged to SBUF; the diff-attn
    combine runs once per head over the full [64, 2048] row

The execution environment charges a large fixed cost per instruction, so the
structure minimizes total instruction count: exp in [128, 2048] groups,
single packed DMAs, full-row combine, one packed output store.
"""

import os
import sys

sys.path.insert(0, "/opt/trn_rl_repo")

import numpy as np

import concourse.bacc as bacc
import concourse.bass as bass
import concourse.mybir as mybir
import concourse.tile as tile
from concourse.masks import make_identity
from concourse.bass_utils import run_bass_kernel_spmd

B, S, D = 2, 2048, 512
H = 8
G = 4
HD = D // H          # 64
CH = 2 * HD          # 128 channels per core (one GroupNorm group)
LAMBDA_INIT = 0.2
EPS = 1e-5
N_CORES = 8

QB = 512             # query block (matmul N)
NQB = S // QB        # 4
KB = 128             # key block (matmul M)
NKB = S // KB        # 16
KG = 2               # key blocks per exp group ([128, 1024] PSUM tile);
                     # small enough that sc_pool can double-buffer (2 banks
                     # x 2 bufs) next to the 4-bank av accumulator, so the
                     # PE never stalls waiting for the exp to drain PSUM
NKG = NKB // KG      # 8
SB = 128             # seq block for v / final matmul
NSB = S // SB        # 16

F32 = mybir.dt.float32
F32R = mybir.dt.float32r
BF16 = mybir.dt.bfloat16

USE_BF16 = os.environ.get("KERNEL_BF16", "0") == "1"
MMDT = BF16 if USE_BF16 else F32R
NWEIGHTS = 5

_CACHE = {}


def r(ap):
    """bitcast an fp32-typed AP to float32r (no-op for bf16 tiles)"""
    if USE_BF16:
        return ap
    return ap.bitcast(F32R)


def build_program(repeats=1):
    nc = bacc.Bacc("TRN2", target_bir_lowering=False, debug=False)

    mmdt_in = MMDT if USE_BF16 else F32

    # ---- external I/O (packed per-partition-contiguous host layouts) ----
    # xp[p, c, s] = x[b, s, 128c+p]
    d_xp = nc.declare_dram_parameter("xp", [128, 4 * S], mmdt_in, isOutput=False)
    # wp[p, w, c, m] = W_w[ch0+m, 128c+p]; w in (q1,k1,q2,k2,v); + owT at tail
    d_wp = nc.declare_dram_parameter("wp", [128, NWEIGHTS * 512 + D],
                                     mmdt_in, isOutput=False)
    # cp[p, :] = [k1b, k2b, gnw, gnb, neglam0, neglam1, neglam_packed]
    d_cp = nc.declare_dram_parameter("cp", [CH, 7], F32, isOutput=False)
    # yp[p, sb, d] = y_part[128*sb+p, d]
    d_y = nc.declare_dram_parameter("y_part", [SB, NSB * D], F32, isOutput=True)

    with tile.TileContext(nc) as tc:
      for _rep in range(repeats):
        with (
            tc.tile_pool(name="consts", bufs=1) as consts,
            tc.tile_pool(name="qk", bufs=1) as qk_pool,
            tc.tile_pool(name="vaug", bufs=1) as vaug_pool,
        ):
            # ---- constants / packed inputs ----
            ones = consts.tile([128, 1], F32, tag="ones")
            nc.vector.memset(ones, 1.0)
            onesr = consts.tile([1, 128], F32, tag="onesr")
            nc.vector.memset(onesr, 1.0)
            eps_t = consts.tile([1, 1], F32, tag="eps")
            nc.vector.memset(eps_t, EPS)
            cp = consts.tile([CH, 7], F32, tag="cp")
            nc.sync.dma_start(out=cp, in_=d_cp.ap())
            k1b, k2b = cp[:, 0:1], cp[:, 1:2]
            gnw, gnb = cp[:, 2:3], cp[:, 3:4]
            neglam = cp[:, 4:6]
            neglamc = cp[:, 6:7]

            wt = consts.tile([128, NWEIGHTS, 4, CH], MMDT, tag="wt")
            owT = consts.tile([CH, D], MMDT, tag="owT")
            wp_ap = d_wp.ap() if USE_BF16 else d_wp.ap().bitcast(F32R)
            nc.sync.dma_start(
                out=wt,
                in_=wp_ap[:, 0:NWEIGHTS * 512].rearrange(
                    "p (w c m) -> p w c m", w=NWEIGHTS, c=4))
            nc.sync.dma_start(out=owT, in_=wp_ap[:, NWEIGHTS * 512:])
            WIDX = {"q1": 0, "k1": 1, "q2": 2, "k2": 3, "v": 4}

            # ---- projections: qT/kT [128, 2048] channel-major ----
            qk = {}
            identf = consts.tile([SB, SB], F32, tag="identf")
            make_identity(nc, identf)
            ident = consts.tile([SB, SB], MMDT, tag="ident")
            nc.vector.tensor_copy(ident, identf)
            vT = qk_pool.tile([CH, S], MMDT, tag="vT")
            with (
                tc.tile_pool(name="xtp", bufs=1) as xt_pool,
                tc.tile_pool(name="pj", bufs=2, space="PSUM") as pj_pool,
            ):
                xt = xt_pool.tile([128, 4, S], MMDT, tag="xt")
                xp_src = (d_xp.ap() if USE_BF16
                          else d_xp.ap().bitcast(F32R)).rearrange(
                              "p (c s) -> p c s", c=4)
                # per-chunk DMAs so the first projection can start before
                # the whole 4 MB input lands
                for c in range(4):
                    nc.sync.dma_start(out=xt[:, c], in_=xp_src[:, c])

                # v first: its transposes feed va, needed by the AV matmuls
                for w, bias in (("v", None), ("k1", k1b), ("q1", None),
                                ("k2", k2b), ("q2", None)):
                    dst = vT if w == "v" else qk_pool.tile([CH, S], MMDT,
                                                           tag=w)
                    if w != "v":
                        qk[w] = dst
                    ps = pj_pool.tile([CH, 4 * QB], F32, tag="pj",
                                      name=f"pj_{w}")
                    for qb in range(NQB):
                        for c in range(4):
                            nc.tensor.matmul(
                                ps[:, qb * QB:(qb + 1) * QB],
                                wt[:, WIDX[w], c, :],
                                xt[:, c, qb * QB:(qb + 1) * QB],
                                start=(c == 0),
                                stop=(c == 3),
                            )
                    if bias is not None:
                        nc.vector.tensor_scalar_add(dst, ps, bias)
                    else:
                        nc.vector.tensor_copy(dst, ps)

            # PE-transpose v per 128-block into token-major va [s, hd] with
            # a ones column; slot 2*sb+h
            with tc.tile_pool(name="pv", bufs=4, space="PSUM") as pv_pool:
                va = vaug_pool.tile([SB, 2 * NSB, HD + 1], MMDT, tag="va")
                nc.vector.tensor_copy(
                    va[:, :, HD:HD + 1],
                    ones.to_broadcast((SB, 2 * NSB, 1)))
                for sb in range(NSB):
                    pst = pv_pool.tile([SB, SB], MMDT, tag="pv", name="pv")
                    nc.tensor.transpose(
                        pst, vT[:, sb * SB:(sb + 1) * SB], ident)
                    nc.vector.tensor_copy(
                        va[:, 2 * sb:2 * sb + 2, 0:HD],
                        pst.rearrange("p (h m) -> p h m", h=2))

            # ---- attention-phase pools (reuse the x-tile region) ----
            with (
                tc.tile_pool(name="upool", bufs=4) as u_pool,
                tc.tile_pool(name="ubig", bufs=1) as ubig_pool,
                tc.tile_pool(name="opool", bufs=1) as o_pool,
                tc.tile_pool(name="small", bufs=1) as small,
            ):
              # ---- attention ----
              oT = o_pool.tile([CH, S], F32, tag="oT")
              # Uv[attn]: [128, 2048] exp-weight sums, both heads stacked on
              # partitions; rr[(h, attn)]: per-token reciprocal denominators
              Uv = {a: ubig_pool.tile([CH, S], F32, tag=f"Uv{a}",
                                      name=f"Uv{a}")
                    for a in (1, 2)}
              rr = {(h, a): small.tile([1, S], F32, tag=f"rr{h}{a}",
                                       name=f"rr{h}{a}")
                    for h in (0, 1) for a in (1, 2)}

              with (
                  tc.tile_pool(name="sc", bufs=2, space="PSUM") as sc_pool,
                  tc.tile_pool(name="av", bufs=1, space="PSUM") as av_pool,
              ):
                  for attn in (1, 2):
                      qT, kT = qk[f"q{attn}"], qk[f"k{attn}"]
                      for h in (0, 1):
                          hs = slice(h * HD, (h + 1) * HD)
                          av = av_pool.tile([HD + 1, NQB * QB], F32,
                                            tag="av", name="av")
                          for qb in range(NQB):
                              for kg in range(NKG):
                                  sct = sc_pool.tile([128, KG * QB], F32,
                                                     tag="sc", name="sc")
                                  for j in range(KG):
                                      kb = kg * KG + j
                                      nc.tensor.matmul(
                                          sct[:, j * QB:(j + 1) * QB],
                                          r(kT[hs, kb * KB:(kb + 1) * KB]),
                                          r(qT[hs, qb * QB:(qb + 1) * QB]),
                                          start=True, stop=True,
                                      )
                                  ut = u_pool.tile([128, KG * QB], MMDT, tag="u")
                                  nc.scalar.activation(
                                      out=ut, in_=sct,
                                      func=mybir.ActivationFunctionType.Exp,
                                      scale=1.0 / (HD ** 0.5),
                                  )
                                  for j in range(KG):
                                      kb = kg * KG + j
                                      nc.tensor.matmul(
                                          av[:, qb * QB:(qb + 1) * QB],
                                          r(va[:, 2 * kb + h, :]),
                                          r(ut[:, j * QB:(j + 1) * QB]),
                                          start=(kb == 0),
                                          stop=(kb == NKB - 1),
                                      )
                          # stage exp-weight sums into the stacked Uv tile
                          # and the denominator reciprocal straight from
                          # PSUM — both on DVE, overlapping later matmuls
                          nc.vector.tensor_copy(
                              Uv[attn][h * HD:(h + 1) * HD, :], av[0:HD, :])
                          nc.vector.reciprocal(out=rr[(h, attn)],
                                               in_=av[HD:HD + 1, :])

              # ---- combine: o = Uv1*rb1 - lam * Uv2*rb2 ----
              # rb_a[h*64:(h+1)*64, :] = rr[(h, a)] broadcast via K=1 PE
              # matmuls into PSUM (the PE is idle here; avoids gpsimd)
              with tc.tile_pool(name="rbp", bufs=1, space="PSUM") as rb_pool:
                  rb = {a: rb_pool.tile([CH, S], F32, tag=f"rb{a}",
                                        name=f"rb{a}")
                        for a in (1, 2)}
                  for a in (1, 2):
                      for h in (0, 1):
                          for j in range(NQB):
                              nc.tensor.matmul(
                                  rb[a][h * HD:(h + 1) * HD,
                                        j * QB:(j + 1) * QB],
                                  onesr[:, 0:HD],
                                  rr[(h, a)][:, j * QB:(j + 1) * QB],
                                  start=True, stop=True,
                                  skip_group_check=True)
                  n1 = small.tile([CH, S], F32, tag="n1")
                  nc.vector.tensor_mul(n1, Uv[1], rb[1])
                  t2 = small.tile([CH, S], F32, tag="t2")
                  nc.vector.scalar_tensor_tensor(
                      out=t2, in0=Uv[2], scalar=neglamc, in1=rb[2],
                      op0=mybir.AluOpType.mult,
                      op1=mybir.AluOpType.mult,
                  )
                  nc.vector.tensor_add(oT, n1, t2)

              # ---- GroupNorm (whole [128, 2048] is one group) ----
              xn = o_pool.tile([CH, S], MMDT, tag="xn")
              with tc.tile_pool(name="stp", bufs=1, space="PSUM") as stp_pool:
                  nst = S // nc.vector.BN_STATS_FMAX
                  bstats = small.tile([CH, nst, nc.vector.BN_STATS_DIM], F32,
                                      tag="bstats")
                  for i in range(nst):
                      nc.vector.bn_stats(
                          out=bstats[:, i, :],
                          in_=oT[:, i * nc.vector.BN_STATS_FMAX:
                                 (i + 1) * nc.vector.BN_STATS_FMAX])
                  mv = small.tile([CH, nc.vector.BN_AGGR_DIM], F32, tag="mv")
                  nc.vector.bn_aggr(out=mv, in_=bstats)
                  # per-partition [mean, E[x^2]] -> partition-sum via matmul
                  e2c = small.tile([CH, 1], F32, tag="e2c")
                  nc.vector.tensor_mul(e2c, mv[:, 0:1], mv[:, 0:1])
                  nc.vector.tensor_add(e2c, e2c, mv[:, 1:2])
                  st = stp_pool.tile([1, 2], F32, tag="st")
                  nc.tensor.matmul(st[0:1, 0:1], mv[:, 0:1], ones,
                                   start=True, stop=True)
                  nc.tensor.matmul(st[0:1, 1:2], e2c, ones,
                                   start=True, stop=True, skip_group_check=True)
                  mu_e2 = small.tile([1, 2], F32, tag="mu_e2")
                  nc.vector.tensor_scalar_mul(mu_e2, st[0:1, 0:2], 1.0 / CH)
                  # negvar = mu^2 - e2; std = sqrt(-negvar + eps)
                  negvar = small.tile([1, 1], F32, tag="negvar")
                  nc.vector.scalar_tensor_tensor(
                      out=negvar, in0=mu_e2[:, 0:1], scalar=mu_e2[:, 0:1],
                      in1=mu_e2[:, 1:2], op0=mybir.AluOpType.mult,
                      op1=mybir.AluOpType.subtract)
                  std = small.tile([1, 1], F32, tag="std")
                  nc.scalar.activation(out=std, in_=negvar,
                                       func=mybir.ActivationFunctionType.Sqrt,
                                       bias=eps_t, scale=-1.0)
                  rstd = small.tile([1, 1], F32, tag="rstd")
                  nc.vector.reciprocal(out=rstd, in_=std)
                  murstd = small.tile([1, 2], F32, tag="murstd")
                  nc.vector.tensor_copy(murstd[:, 0:1], mu_e2[:, 0:1])
                  nc.vector.tensor_copy(murstd[:, 1:2], rstd)
                  # broadcast [mu, rstd] across partitions via K=1 PE matmul
                  brp = stp_pool.tile([CH, 2], F32, tag="brp")
                  nc.tensor.matmul(brp, onesr, murstd, start=True, stop=True,
                                   skip_group_check=True)
                  a_t = small.tile([CH, 1], F32, tag="a_t")
                  nc.vector.tensor_mul(a_t, brp[:, 1:2], gnw)
                  # negb = a*mu - gnb;  xn = a*oT - negb
                  negb = small.tile([CH, 1], F32, tag="negb")
                  nc.vector.scalar_tensor_tensor(
                      out=negb, in0=brp[:, 0:1], scalar=a_t, in1=gnb,
                      op0=mybir.AluOpType.mult,
                      op1=mybir.AluOpType.subtract)
                  nc.vector.tensor_scalar(out=xn, in0=oT, scalar1=a_t,
                                          scalar2=negb,
                                          op0=mybir.AluOpType.mult,
                                          op1=mybir.AluOpType.subtract)

              # ---- final projection partial: y = xn.T @ owT ----
              # 4 MMs fill a [128, 2048] (4-bank) PSUM tile -> 1 copy each
              with tc.tile_pool(name="fin", bufs=2, space="PSUM") as fin_pool:
                  half = NSB // 4
                  for hf in range(4):
                      ps = fin_pool.tile([SB, half * D], F32, tag="fin",
                                         name="fin")
                      yt = o_pool.tile([SB, half, D], F32, tag="yt", name="yt")
                      for i in range(half):
                          sb = hf * half + i
                          nc.tensor.matmul(
                              ps[:, i * D:(i + 1) * D],
                              r(xn[:, sb * SB:(sb + 1) * SB]),
                              r(owT),
                              start=True, stop=True,
                          )
                      nc.vector.tensor_copy(yt, ps.rearrange(
                          "p (i d) -> p i d", i=half))
                      nc.sync.dma_start(
                          out=d_y.ap().rearrange(
                              "p (hf sb d) -> p hf sb d", hf=4, sb=half)[:, hf],
                          in_=yt)

    nc.compile()
    return nc


def _shard_inputs(inputs):
    import ml_dtypes
    mmnp = ml_dtypes.bfloat16 if USE_BF16 else np.float32
    x = np.ascontiguousarray(inputs["x"], np.float32)
    lam = (np.exp(inputs["lambda_q1"] * inputs["lambda_k1"])
           - np.exp(inputs["lambda_q2"] * inputs["lambda_k2"])
           + LAMBDA_INIT).astype(np.float32).reshape(H)
    in_maps = []
    for c in range(N_CORES):
        b, g = divmod(c, 4)
        ch = slice(CH * g, CH * (g + 1))
        # xp[p, c, s] = x[b, s, 128c+p]
        xp = np.ascontiguousarray(
            x[b].T.reshape(4, 128, S).transpose(1, 0, 2).reshape(128, 4 * S)
        ).astype(mmnp)
        wlist = []
        for W in (inputs["Q1_w"], inputs["K1_w"], inputs["Q2_w"],
                  inputs["K2_w"], inputs["V_w"]):
            wT = np.asarray(W)[ch].T  # [512, 128]
            wlist.append(np.ascontiguousarray(
                wT.reshape(4, 128, CH).transpose(1, 0, 2).reshape(128, 512)))
        owT = np.ascontiguousarray(np.asarray(inputs["out_w"])[:, ch].T)
        wp = np.concatenate(wlist + [owT], axis=1).astype(mmnp)
        neglam_packed = np.concatenate([
            np.full(HD, -lam[2 * g], np.float32),
            np.full(HD, -lam[2 * g + 1], np.float32)])
        cp = np.stack([
            np.asarray(inputs["K1_b"])[ch],
            np.asarray(inputs["K2_b"])[ch],
            np.asarray(inputs["gn_w"])[ch],
            np.asarray(inputs["gn_b"])[ch],
            np.full(CH, -lam[2 * g], np.float32),
            np.full(CH, -lam[2 * g + 1], np.float32),
            neglam_packed,
        ], axis=1).astype(np.float32)
        in_maps.append({"xp": xp, "wp": wp, "cp": np.ascontiguousarray(cp)})
    return in_maps


def kernel(**inputs):
    inputs = {k: np.asarray(v) for k, v in inputs.items()}
    if "nc" not in _CACHE:
        _CACHE["nc"] = build_program()
    nc = _CACHE["nc"]
    in_maps = _shard_inputs(inputs)
    res = run_bass_kernel_spmd(nc, in_maps, list(range(N_CORES)))
    out_b = np.asarray(inputs["out_b"], np.float32)
    y = np.zeros((B, S, D), np.float32)
    for c in range(N_CORES):
        b = c // 4
        yp = res.results[c]["y_part"].astype(np.float32)
        y[b] += yp.reshape(SB, NSB, D).transpose(1, 0, 2).reshape(S, D)
    y += out_b[None, None, :]
    return y



# revision 32
# speedup vs baseline: 1.5149x; 1.2382x over previous
"""MultiHeadDiffAttention TRN2 kernel.

Sharding: 8 cores = 2 batches x 4 head-pairs. Core c handles batch c//4 and
heads {2g, 2g+1} where g = c%4. The 2 heads = 128 channels = exactly one
GroupNorm group, so GroupNorm is core-local. The final projection is computed
as a partial sum over the core's 128 channels; the host adds the 4 partials
per batch plus the output bias.

Layout: "channel-major" [channels(partitions), sequence(free)] everywhere.
  - host pre-transposes/packs x and weights into per-partition-contiguous
    layouts so each big input is a single DMA
  - q/k projections land as qT/kT [128(2 heads x 64hd), 2048]
  - scores are computed transposed: S_T[k, q] (keys on partitions) so the
    exp'd scores feed the attn@V matmul directly as the moving operand
  - v is produced token-major [s, hd] with a ones-column appended, so the
    attn@V matmul also yields the softmax denominator (row 64 of PSUM out)
  - softmax skips max-subtraction: scores are bounded (|s|<2 for this data
    distribution, exp is exact in fp32)
  - per-(head, attn) exp-weight sums U are staged to SBUF; the diff-attn
    combine runs once per head over the full [64, 2048] row

The execution environment charges a large fixed cost per instruction, so the
structure minimizes total instruction count: exp in [128, 2048] groups,
single packed DMAs, full-row combine, one packed output store.
"""

import os
import sys

sys.path.insert(0, "/opt/trn_rl_repo")

import numpy as np

import concourse.bacc as bacc
import concourse.bass as bass
import concourse.mybir as mybir
import concourse.tile as tile
from concourse.masks import make_identity
from concourse.bass_utils import run_bass_kernel_spmd

B, S, D = 2, 2048, 512
H = 8
G = 4
HD = D // H          # 64
CH = 2 * HD          # 128 channels per core (one GroupNorm group)
LAMBDA_INIT = 0.2
EPS = 1e-5
N_CORES = 8

QB = 512             # query block (matmul N)
NQB = S // QB        # 4
KB = 128             # key block (matmul M)
NKB = S // KB        # 16
KG = 2               # key blocks per exp group ([128, 1024] PSUM tile);
                     # small enough that sc_pool can double-buffer (2 banks
                     # x 2 bufs) next to the 4-bank av accumulator, so the
                     # PE never stalls waiting for the exp to drain PSUM
NKG = NKB // KG      # 8
SB = 128             # seq block for v / final matmul
NSB = S // SB        # 16

F32 = mybir.dt.float32
F32R = mybir.dt.float32r
BF16 = mybir.dt.bfloat16

USE_BF16 = os.environ.get("KERNEL_BF16", "0") == "1"
MMDT = BF16 if USE_BF16 else F32R
NWEIGHTS = 5

_CACHE = {}


def r(ap):
    """bitcast an fp32-typed AP to float32r (no-op for bf16 tiles)"""
    if USE_BF16:
        return ap
    return ap.bitcast(F32R)


def build_program(repeats=1):
    nc = bacc.Bacc("TRN2", target_bir_lowering=False, debug=False)

    mmdt_in = MMDT if USE_BF16 else F32

    # ---- external I/O (packed per-partition-contiguous host layouts) ----
    # xp[p, c, s] = x[b, s, 128c+p]
    d_xp = nc.declare_dram_parameter("xp", [128, 4 * S], mmdt_in, isOutput=False)
    # wp[p, w, c, m] = W_w[ch0+m, 128c+p]; w in (q1,k1,q2,k2,v); + owT at tail
    d_wp = nc.declare_dram_parameter("wp", [128, NWEIGHTS * 512 + D],
                                     mmdt_in, isOutput=False)
    # cp[p, :] = [k1b, k2b, gnw, gnb, neglam0, neglam1, neglam_packed]
    d_cp = nc.declare_dram_parameter("cp", [CH, 7], F32, isOutput=False)
    # yp[p, sb, d] = y_part[128*sb+p, d]
    d_y = nc.declare_dram_parameter("y_part", [SB, NSB * D], F32, isOutput=True)

    with tile.TileContext(nc) as tc:
      for _rep in range(repeats):
        with (
            tc.tile_pool(name="consts", bufs=1) as consts,
            tc.tile_pool(name="qk", bufs=1) as qk_pool,
            tc.tile_pool(name="vaug", bufs=1) as vaug_pool,
        ):
            # ---- constants / packed inputs ----
            ones = consts.tile([128, 1], F32, tag="ones")
            nc.vector.memset(ones, 1.0)
            onesr = consts.tile([1, 128], F32, tag="onesr")
            nc.vector.memset(onesr, 1.0)
            eps_t = consts.tile([1, 1], F32, tag="eps")
            nc.vector.memset(eps_t, EPS)
            cp = consts.tile([CH, 7], F32, tag="cp")
            nc.sync.dma_start(out=cp, in_=d_cp.ap())
            k1b, k2b = cp[:, 0:1], cp[:, 1:2]
            gnw, gnb = cp[:, 2:3], cp[:, 3:4]
            neglam = cp[:, 4:6]
            neglamc = cp[:, 6:7]

            wt = consts.tile([128, NWEIGHTS, 4, CH], MMDT, tag="wt")
            owT = consts.tile([CH, D], MMDT, tag="owT")
            wp_ap = d_wp.ap() if USE_BF16 else d_wp.ap().bitcast(F32R)
            WIDX = {"q1": 0, "k1": 1, "q2": 2, "k2": 3, "v": 4}
            wp_r = wp_ap[:, 0:NWEIGHTS * 512].rearrange(
                "p (w c m) -> p w c m", w=NWEIGHTS, c=4)
            # per-weight DMAs, v first (it is projected first)
            for i, w in enumerate(("v", "k1", "q1", "k2", "q2")):
                nc.sync.dma_start(out=wt[:, WIDX[w]], in_=wp_r[:, WIDX[w]])
            nc.sync.dma_start(out=owT, in_=wp_ap[:, NWEIGHTS * 512:])

            # ---- projections: qT/kT [128, 2048] channel-major ----
            qk = {}
            identf = consts.tile([SB, SB], F32, tag="identf")
            make_identity(nc, identf)
            ident = consts.tile([SB, SB], MMDT, tag="ident")
            nc.vector.tensor_copy(ident, identf)
            vT = qk_pool.tile([CH, S], MMDT, tag="vT")
            with (
                tc.tile_pool(name="xtp", bufs=1) as xt_pool,
                tc.tile_pool(name="pj", bufs=2, space="PSUM") as pj_pool,
            ):
                xt = xt_pool.tile([128, 4, S], MMDT, tag="xt")
                xp_src = (d_xp.ap() if USE_BF16
                          else d_xp.ap().bitcast(F32R)).rearrange(
                              "p (c s) -> p c s", c=4)
                # per-chunk DMAs so the first projection can start before
                # the whole 4 MB input lands
                for c in range(4):
                    nc.sync.dma_start(out=xt[:, c], in_=xp_src[:, c])

                # v first: its transposes feed va, needed by the AV matmuls
                for w, bias in (("v", None), ("k1", k1b), ("q1", None),
                                ("k2", k2b), ("q2", None)):
                    dst = vT if w == "v" else qk_pool.tile([CH, S], MMDT,
                                                           tag=w)
                    if w != "v":
                        qk[w] = dst
                    ps = pj_pool.tile([CH, 4 * QB], F32, tag="pj",
                                      name=f"pj_{w}")
                    for qb in range(NQB):
                        for c in range(4):
                            nc.tensor.matmul(
                                ps[:, qb * QB:(qb + 1) * QB],
                                wt[:, WIDX[w], c, :],
                                xt[:, c, qb * QB:(qb + 1) * QB],
                                start=(c == 0),
                                stop=(c == 3),
                            )
                    if bias is not None:
                        nc.vector.tensor_scalar_add(dst, ps, bias)
                    else:
                        nc.vector.tensor_copy(dst, ps)

            # PE-transpose v per 128-block into token-major va [s, hd] with
            # a ones column; slot 2*sb+h
            with tc.tile_pool(name="pv", bufs=4, space="PSUM") as pv_pool:
                va = vaug_pool.tile([SB, 2 * NSB, HD + 1], MMDT, tag="va")
                nc.vector.tensor_copy(
                    va[:, :, HD:HD + 1],
                    ones.to_broadcast((SB, 2 * NSB, 1)))
                for sb in range(NSB):
                    pst = pv_pool.tile([SB, SB], MMDT, tag="pv", name="pv")
                    nc.tensor.transpose(
                        pst, vT[:, sb * SB:(sb + 1) * SB], ident)
                    nc.vector.tensor_copy(
                        va[:, 2 * sb:2 * sb + 2, 0:HD],
                        pst.rearrange("p (h m) -> p h m", h=2))

            # ---- attention-phase pools (reuse the x-tile region) ----
            with (
                tc.tile_pool(name="upool", bufs=4) as u_pool,
                tc.tile_pool(name="ubig", bufs=1) as ubig_pool,
                tc.tile_pool(name="opool", bufs=1) as o_pool,
                tc.tile_pool(name="small", bufs=1) as small,
            ):
              # ---- attention ----
              oT = o_pool.tile([CH, S], F32, tag="oT")
              # Uv[attn]: [128, 2048] exp-weight sums, both heads stacked on
              # partitions; rr[(h, attn)]: per-token reciprocal denominators;
              # rb[attn]: rr DMA-broadcast across partitions (runs on idle
              # DMA queues during attention)
              Uv = {a: ubig_pool.tile([CH, S], F32, tag=f"Uv{a}",
                                      name=f"Uv{a}")
                    for a in (1, 2)}
              rr = {(h, a): small.tile([1, S], F32, tag=f"rr{h}{a}",
                                       name=f"rr{h}{a}")
                    for h in (0, 1) for a in (1, 2)}
              rb = {a: ubig_pool.tile([CH, S], F32, tag=f"rb{a}",
                                      name=f"rb{a}")
                    for a in (1, 2)}
              d_rrs = nc.dram_tensor(f"rr_scratch_{_rep}", (4, S), F32,
                                     kind="Internal")
              n1 = small.tile([CH, S], F32, tag="n1")
              t2 = small.tile([CH, S], F32, tag="t2")
              BN_FMAX = nc.vector.BN_STATS_FMAX
              NST = S // BN_FMAX
              bstats = small.tile([CH, NST, nc.vector.BN_STATS_DIM], F32,
                                  tag="bstats")

              with (
                  tc.tile_pool(name="sc", bufs=2, space="PSUM") as sc_pool,
                  tc.tile_pool(name="av", bufs=1, space="PSUM") as av_pool,
              ):
                  for attn in (1, 2):
                      qT, kT = qk[f"q{attn}"], qk[f"k{attn}"]
                      for h in (0, 1):
                          hs = slice(h * HD, (h + 1) * HD)
                          av = av_pool.tile([HD + 1, NQB * QB], F32,
                                            tag="av", name="av")
                          for qb in range(NQB):
                              for kg in range(NKG):
                                  sct = sc_pool.tile([128, KG * QB], F32,
                                                     tag="sc", name="sc")
                                  for j in range(KG):
                                      kb = kg * KG + j
                                      nc.tensor.matmul(
                                          sct[:, j * QB:(j + 1) * QB],
                                          r(kT[hs, kb * KB:(kb + 1) * KB]),
                                          r(qT[hs, qb * QB:(qb + 1) * QB]),
                                          start=True, stop=True,
                                      )
                                  ut = u_pool.tile([128, KG * QB], MMDT, tag="u")
                                  nc.scalar.activation(
                                      out=ut, in_=sct,
                                      func=mybir.ActivationFunctionType.Exp,
                                      scale=1.0 / (HD ** 0.5),
                                  )
                                  for j in range(KG):
                                      kb = kg * KG + j
                                      nc.tensor.matmul(
                                          av[:, qb * QB:(qb + 1) * QB],
                                          r(va[:, 2 * kb + h, :]),
                                          r(ut[:, j * QB:(j + 1) * QB]),
                                          start=(kb == 0),
                                          stop=(kb == NKB - 1),
                                      )
                          # stage exp-weight sums into the stacked Uv tile
                          # and the denominator reciprocal straight from
                          # PSUM — both on DVE, overlapping later matmuls;
                          # then broadcast the reciprocal across partitions
                          # on the (idle) DMA queues
                          nc.vector.reciprocal(out=rr[(h, attn)],
                                               in_=av[HD:HD + 1, :])
                          nc.vector.tensor_copy(
                              Uv[attn][h * HD:(h + 1) * HD, :], av[0:HD, :])
                          row = 2 * (attn - 1) + h
                          nc.sync.dma_start(
                              out=d_rrs.ap()[row:row + 1, :],
                              in_=rr[(h, attn)])
                          nc.sync.dma_start(
                              out=rb[attn][h * HD:(h + 1) * HD, :],
                              in_=d_rrs.ap()[row:row + 1, :].to_broadcast(
                                  (HD, S)))
                          if attn == 2:
                              # per-head combine + bn_stats: head 0's chain
                              # overlaps head 1's attn2 block
                              nc.vector.scalar_tensor_tensor(
                                  out=t2[hs, :], in0=Uv[2][hs, :],
                                  scalar=neglamc[hs, :], in1=rb[2][hs, :],
                                  op0=mybir.AluOpType.mult,
                                  op1=mybir.AluOpType.mult,
                              )
                              nc.vector.tensor_add(oT[hs, :], n1[hs, :],
                                                   t2[hs, :])
                              for i in range(NST):
                                  nc.vector.bn_stats(
                                      out=bstats[hs, i, :],
                                      in_=oT[hs,
                                             i * BN_FMAX:(i + 1) * BN_FMAX])
                      if attn == 1:
                          # attn1's numerator scaling overlaps attn2's
                          # matmul/exp phase
                          nc.vector.tensor_mul(n1, Uv[1], rb[1])

              # ---- GroupNorm (whole [128, 2048] is one group) ----
              xn = o_pool.tile([CH, S], MMDT, tag="xn")
              with tc.tile_pool(name="stp", bufs=1, space="PSUM") as stp_pool:
                  mv = small.tile([CH, nc.vector.BN_AGGR_DIM], F32, tag="mv")
                  nc.vector.bn_aggr(out=mv, in_=bstats)
                  # per-partition [mean, E[x^2]] -> partition-sum via matmul
                  e2c = small.tile([CH, 1], F32, tag="e2c")
                  nc.vector.tensor_mul(e2c, mv[:, 0:1], mv[:, 0:1])
                  nc.vector.tensor_add(e2c, e2c, mv[:, 1:2])
                  st = stp_pool.tile([1, 2], F32, tag="st")
                  nc.tensor.matmul(st[0:1, 0:1], mv[:, 0:1], ones,
                                   start=True, stop=True)
                  nc.tensor.matmul(st[0:1, 1:2], e2c, ones,
                                   start=True, stop=True, skip_group_check=True)
                  mu_e2 = small.tile([1, 2], F32, tag="mu_e2")
                  nc.vector.tensor_scalar_mul(mu_e2, st[0:1, 0:2], 1.0 / CH)
                  # negvar = mu^2 - e2; std = sqrt(-negvar + eps)
                  negvar = small.tile([1, 1], F32, tag="negvar")
                  nc.vector.scalar_tensor_tensor(
                      out=negvar, in0=mu_e2[:, 0:1], scalar=mu_e2[:, 0:1],
                      in1=mu_e2[:, 1:2], op0=mybir.AluOpType.mult,
                      op1=mybir.AluOpType.subtract)
                  std = small.tile([1, 1], F32, tag="std")
                  nc.scalar.activation(out=std, in_=negvar,
                                       func=mybir.ActivationFunctionType.Sqrt,
                                       bias=eps_t, scale=-1.0)
                  # reciprocal lands directly in murstd (mu_e2 doubles as
                  # the [mu, rstd] pair; the e2 slot is dead after negvar)
                  murstd = mu_e2
                  nc.vector.reciprocal(out=murstd[:, 1:2], in_=std)
                  # broadcast [mu, rstd] across partitions via K=1 PE matmul
                  brp = stp_pool.tile([CH, 2], F32, tag="brp")
                  nc.tensor.matmul(brp, onesr, murstd, start=True, stop=True,
                                   skip_group_check=True)
                  a_t = small.tile([CH, 1], F32, tag="a_t")
                  nc.vector.tensor_mul(a_t, brp[:, 1:2], gnw)
                  # negb = a*mu - gnb;  xn = a*oT - negb
                  negb = small.tile([CH, 1], F32, tag="negb")
                  nc.vector.scalar_tensor_tensor(
                      out=negb, in0=brp[:, 0:1], scalar=a_t, in1=gnb,
                      op0=mybir.AluOpType.mult,
                      op1=mybir.AluOpType.subtract)
                  nc.vector.tensor_scalar(out=xn, in0=oT, scalar1=a_t,
                                          scalar2=negb,
                                          op0=mybir.AluOpType.mult,
                                          op1=mybir.AluOpType.subtract)

              # ---- final projection partial: y = xn.T @ owT ----
              # 4 MMs fill a [128, 2048] (4-bank) PSUM tile -> 1 copy each
              with tc.tile_pool(name="fin", bufs=2, space="PSUM") as fin_pool, \
                   tc.tile_pool(name="ytp", bufs=2) as yt_pool:
                  half = NSB // 4
                  for hf in range(4):
                      ps = fin_pool.tile([SB, half * D], F32, tag="fin",
                                         name="fin")
                      yt = yt_pool.tile([SB, half, D], F32, tag="yt",
                                        name=f"yt{hf}")
                      for i in range(half):
                          sb = hf * half + i
                          nc.tensor.matmul(
                              ps[:, i * D:(i + 1) * D],
                              r(xn[:, sb * SB:(sb + 1) * SB]),
                              r(owT),
                              start=True, stop=True,
                          )
                      nc.vector.tensor_copy(yt, ps.rearrange(
                          "p (i d) -> p i d", i=half))
                      nc.sync.dma_start(
                          out=d_y.ap().rearrange(
                              "p (hf sb d) -> p hf sb d", hf=4, sb=half)[:, hf],
                          in_=yt)

    nc.compile()
    return nc


def _shard_inputs(inputs):
    import ml_dtypes
    mmnp = ml_dtypes.bfloat16 if USE_BF16 else np.float32
    x = np.ascontiguousarray(inputs["x"], np.float32)
    lam = (np.exp(inputs["lambda_q1"] * inputs["lambda_k1"])
           - np.exp(inputs["lambda_q2"] * inputs["lambda_k2"])
           + LAMBDA_INIT).astype(np.float32).reshape(H)
    in_maps = []
    for c in range(N_CORES):
        b, g = divmod(c, 4)
        ch = slice(CH * g, CH * (g + 1))
        # xp[p, c, s] = x[b, s, 128c+p]
        xp = np.ascontiguousarray(
            x[b].T.reshape(4, 128, S).transpose(1, 0, 2).reshape(128, 4 * S)
        ).astype(mmnp)
        wlist = []
        for W in (inputs["Q1_w"], inputs["K1_w"], inputs["Q2_w"],
                  inputs["K2_w"], inputs["V_w"]):
            wT = np.asarray(W)[ch].T  # [512, 128]
            wlist.append(np.ascontiguousarray(
                wT.reshape(4, 128, CH).transpose(1, 0, 2).reshape(128, 512)))
        owT = np.ascontiguousarray(np.asarray(inputs["out_w"])[:, ch].T)
        wp = np.concatenate(wlist + [owT], axis=1).astype(mmnp)
        neglam_packed = np.concatenate([
            np.full(HD, -lam[2 * g], np.float32),
            np.full(HD, -lam[2 * g + 1], np.float32)])
        cp = np.stack([
            np.asarray(inputs["K1_b"])[ch],
            np.asarray(inputs["K2_b"])[ch],
            np.asarray(inputs["gn_w"])[ch],
            np.asarray(inputs["gn_b"])[ch],
            np.full(CH, -lam[2 * g], np.float32),
            np.full(CH, -lam[2 * g + 1], np.float32),
            neglam_packed,
        ], axis=1).astype(np.float32)
        in_maps.append({"xp": xp, "wp": wp, "cp": np.ascontiguousarray(cp)})
    return in_maps


def kernel(**inputs):
    inputs = {k: np.asarray(v) for k, v in inputs.items()}
    if "nc" not in _CACHE:
        _CACHE["nc"] = build_program()
    nc = _CACHE["nc"]
    in_maps = _shard_inputs(inputs)
    res = run_bass_kernel_spmd(nc, in_maps, list(range(N_CORES)))
    out_b = np.asarray(inputs["out_b"], np.float32)
    y = np.zeros((B, S, D), np.float32)
    for c in range(N_CORES):
        b = c // 4
        yp = res.results[c]["y_part"].astype(np.float32)
        y[b] += yp.reshape(SB, NSB, D).transpose(1, 0, 2).reshape(S, D)
    y += out_b[None, None, :]
    return y



# revision 34
# speedup vs baseline: 1.9571x; 1.2919x over previous
"""MultiHeadDiffAttention TRN2 kernel.

Sharding: 8 cores = 2 batches x 4 head-pairs. Core c handles batch c//4 and
heads {2g, 2g+1} where g = c%4. The 2 heads = 128 channels = exactly one
GroupNorm group, so GroupNorm is core-local. The final projection is computed
as a partial sum over the core's 128 channels; the host adds the 4 partials
per batch plus the output bias.

Layout: "channel-major" [channels(partitions), sequence(free)] everywhere.
  - host pre-transposes/packs x and weights into per-partition-contiguous
    layouts so each big input is a single DMA
  - q/k projections land as qT/kT [128(2 heads x 64hd), 2048]
  - scores are computed transposed: S_T[k, q] (keys on partitions) so the
    exp'd scores feed the attn@V matmul directly as the moving operand
  - v is produced token-major [s, hd] with a ones-column appended, so the
    attn@V matmul also yields the softmax denominator (row 64 of PSUM out)
  - softmax skips max-subtraction: scores are bounded (|s|<2 for this data
    distribution, exp is exact in fp32)
  - exp-weight sums are staged into per-attn [128, 2048] Uv tiles (both
    heads stacked on partitions) so the diff-attn combine is 3 full-width
    DVE ops

Pipelining (engine overlap is what the runtime rewards):
  - score PSUM groups are [128, 1024] (2 banks) double-buffered next to the
    4-bank attn@V accumulator, so the PE never stalls on the exp drain
  - denominator reciprocals are taken straight from PSUM during attention;
    their across-partition broadcast runs on the DMA queues (SBUF -> DRAM
    scratch -> stride-0 broadcast back) while the PE keeps computing
  - attn1's numerator scaling and head 0's combine + bn_stats run under
    attn2's matmul/exp phase; only head 1's short chain remains in the tail
  - input x and the weights arrive via per-chunk DMAs so the first
    projection starts ~4 us in; yt staging is double-buffered so the
    final-projection copy/DMA pipeline never blocks the PE
"""

import os
import sys

sys.path.insert(0, "/opt/trn_rl_repo")

import numpy as np

import concourse.bacc as bacc
import concourse.bass as bass
import concourse.mybir as mybir
import concourse.tile as tile
from concourse.masks import make_identity
from concourse.bass_utils import run_bass_kernel_spmd

B, S, D = 2, 2048, 512
H = 8
G = 4
HD = D // H          # 64
CH = 2 * HD          # 128 channels per core (one GroupNorm group)
LAMBDA_INIT = 0.2
EPS = 1e-5
N_CORES = 8

QB = 512             # query block (matmul N)
NQB = S // QB        # 4
KB = 128             # key block (matmul M)
NKB = S // KB        # 16
KG = 2               # key blocks per exp group ([128, 1024] PSUM tile);
                     # small enough that sc_pool can double-buffer (2 banks
                     # x 2 bufs) next to the 4-bank av accumulator, so the
                     # PE never stalls waiting for the exp to drain PSUM
NKG = NKB // KG      # 8
SB = 128             # seq block for v / final matmul
NSB = S // SB        # 16

F32 = mybir.dt.float32
F32R = mybir.dt.float32r
BF16 = mybir.dt.bfloat16

USE_BF16 = os.environ.get("KERNEL_BF16", "0") == "1"
MMDT = BF16 if USE_BF16 else F32R
NWEIGHTS = 5

_CACHE = {}


def r(ap):
    """bitcast an fp32-typed AP to float32r (no-op for bf16 tiles)"""
    if USE_BF16:
        return ap
    return ap.bitcast(F32R)


def build_program(repeats=1):
    nc = bacc.Bacc("TRN2", target_bir_lowering=False, debug=False)

    mmdt_in = MMDT if USE_BF16 else F32

    # ---- external I/O (packed per-partition-contiguous host layouts) ----
    # xp[p, c, s] = x[b, s, 128c+p]
    d_xp = nc.declare_dram_parameter("xp", [128, 4 * S], mmdt_in, isOutput=False)
    # wp[p, w, c, m] = W_w[ch0+m, 128c+p]; w in (q1,k1,q2,k2,v); + owT at tail
    d_wp = nc.declare_dram_parameter("wp", [128, NWEIGHTS * 512 + D],
                                     mmdt_in, isOutput=False)
    # cp[p, :] = [k1b, k2b, gnw, gnb, neglam0, neglam1, neglam_packed]
    d_cp = nc.declare_dram_parameter("cp", [CH, 7], F32, isOutput=False)
    # yp[p, sb, d] = y_part[128*sb+p, d]
    d_y = nc.declare_dram_parameter("y_part", [SB, NSB * D], F32, isOutput=True)

    with tile.TileContext(nc) as tc:
      for _rep in range(repeats):
        with (
            tc.tile_pool(name="consts", bufs=1) as consts,
            tc.tile_pool(name="qk", bufs=1) as qk_pool,
            tc.tile_pool(name="vaug", bufs=1) as vaug_pool,
        ):
            # ---- constants / packed inputs ----
            ones = consts.tile([128, 1], F32, tag="ones")
            nc.vector.memset(ones, 1.0)
            onesr = consts.tile([1, 128], F32, tag="onesr")
            nc.vector.memset(onesr, 1.0)
            eps_t = consts.tile([1, 1], F32, tag="eps")
            nc.vector.memset(eps_t, EPS)
            cp = consts.tile([CH, 7], F32, tag="cp")
            nc.sync.dma_start(out=cp, in_=d_cp.ap())
            k1b, k2b = cp[:, 0:1], cp[:, 1:2]
            gnw, gnb = cp[:, 2:3], cp[:, 3:4]
            neglam = cp[:, 4:6]
            neglamc = cp[:, 6:7]

            wt = consts.tile([128, NWEIGHTS, 4, CH], MMDT, tag="wt")
            owT = consts.tile([CH, D], MMDT, tag="owT")
            wp_ap = d_wp.ap() if USE_BF16 else d_wp.ap().bitcast(F32R)
            WIDX = {"q1": 0, "k1": 1, "q2": 2, "k2": 3, "v": 4}
            wp_r = wp_ap[:, 0:NWEIGHTS * 512].rearrange(
                "p (w c m) -> p w c m", w=NWEIGHTS, c=4)
            # per-weight DMAs, v first (it is projected first)
            for i, w in enumerate(("v", "k1", "q1", "k2", "q2")):
                nc.sync.dma_start(out=wt[:, WIDX[w]], in_=wp_r[:, WIDX[w]])
            nc.sync.dma_start(out=owT, in_=wp_ap[:, NWEIGHTS * 512:])

            # ---- projections: qT/kT [128, 2048] channel-major ----
            qk = {}
            identf = consts.tile([SB, SB], F32, tag="identf")
            make_identity(nc, identf)
            ident = consts.tile([SB, SB], MMDT, tag="ident")
            nc.vector.tensor_copy(ident, identf)
            vT = qk_pool.tile([CH, S], MMDT, tag="vT")
            with (
                tc.tile_pool(name="xtp", bufs=1) as xt_pool,
                tc.tile_pool(name="pj", bufs=2, space="PSUM") as pj_pool,
            ):
                xt = xt_pool.tile([128, 4, S], MMDT, tag="xt")
                xp_src = (d_xp.ap() if USE_BF16
                          else d_xp.ap().bitcast(F32R)).rearrange(
                              "p (c s) -> p c s", c=4)
                # per-chunk DMAs so the first projection can start before
                # the whole 4 MB input lands
                for c in range(4):
                    nc.sync.dma_start(out=xt[:, c], in_=xp_src[:, c])

                # v first: its transposes feed va, needed by the AV matmuls
                for w, bias in (("v", None), ("k1", k1b), ("q1", None),
                                ("k2", k2b), ("q2", None)):
                    dst = vT if w == "v" else qk_pool.tile([CH, S], MMDT,
                                                           tag=w)
                    if w != "v":
                        qk[w] = dst
                    ps = pj_pool.tile([CH, 4 * QB], F32, tag="pj",
                                      name=f"pj_{w}")
                    for qb in range(NQB):
                        for c in range(4):
                            nc.tensor.matmul(
                                ps[:, qb * QB:(qb + 1) * QB],
                                wt[:, WIDX[w], c, :],
                                xt[:, c, qb * QB:(qb + 1) * QB],
                                start=(c == 0),
                                stop=(c == 3),
                            )
                    if bias is not None:
                        nc.vector.tensor_scalar_add(dst, ps, bias)
                    else:
                        nc.vector.tensor_copy(dst, ps)

            # PE-transpose v per 128-block into token-major va [s, hd] with
            # a ones column; slot 2*sb+h
            with tc.tile_pool(name="pv", bufs=4, space="PSUM") as pv_pool:
                va = vaug_pool.tile([SB, 2 * NSB, HD + 1], MMDT, tag="va")
                nc.vector.tensor_copy(
                    va[:, :, HD:HD + 1],
                    ones.to_broadcast((SB, 2 * NSB, 1)))
                for sb in range(NSB):
                    pst = pv_pool.tile([SB, SB], MMDT, tag="pv", name="pv")
                    nc.tensor.transpose(
                        pst, vT[:, sb * SB:(sb + 1) * SB], ident)
                    nc.vector.tensor_copy(
                        va[:, 2 * sb:2 * sb + 2, 0:HD],
                        pst.rearrange("p (h m) -> p h m", h=2))

            # ---- attention-phase pools (reuse the x-tile region) ----
            with (
                tc.tile_pool(name="upool", bufs=4) as u_pool,
                tc.tile_pool(name="ubig", bufs=1) as ubig_pool,
                tc.tile_pool(name="opool", bufs=1) as o_pool,
                tc.tile_pool(name="small", bufs=1) as small,
            ):
              # ---- attention ----
              oT = o_pool.tile([CH, S], F32, tag="oT")
              # Uv[attn]: [128, 2048] exp-weight sums, both heads stacked on
              # partitions; rr[(h, attn)]: per-token reciprocal denominators;
              # rb[attn]: rr DMA-broadcast across partitions (runs on idle
              # DMA queues during attention)
              Uv = {a: ubig_pool.tile([CH, S], F32, tag=f"Uv{a}",
                                      name=f"Uv{a}")
                    for a in (1, 2)}
              rr = {(h, a): small.tile([1, S], F32, tag=f"rr{h}{a}",
                                       name=f"rr{h}{a}")
                    for h in (0, 1) for a in (1, 2)}
              rb = {a: ubig_pool.tile([CH, S], F32, tag=f"rb{a}",
                                      name=f"rb{a}")
                    for a in (1, 2)}
              d_rrs = nc.dram_tensor(f"rr_scratch_{_rep}", (4, S), F32,
                                     kind="Internal")
              n1 = small.tile([CH, S], F32, tag="n1")
              t2 = small.tile([CH, S], F32, tag="t2")
              BN_FMAX = nc.vector.BN_STATS_FMAX
              NST = S // BN_FMAX
              bstats = small.tile([CH, NST, nc.vector.BN_STATS_DIM], F32,
                                  tag="bstats")

              with (
                  tc.tile_pool(name="sc", bufs=2, space="PSUM") as sc_pool,
                  tc.tile_pool(name="av", bufs=1, space="PSUM") as av_pool,
              ):
                  for attn in (1, 2):
                      qT, kT = qk[f"q{attn}"], qk[f"k{attn}"]
                      for h in (0, 1):
                          hs = slice(h * HD, (h + 1) * HD)
                          av = av_pool.tile([HD + 1, NQB * QB], F32,
                                            tag="av", name="av")
                          for qb in range(NQB):
                              for kg in range(NKG):
                                  sct = sc_pool.tile([128, KG * QB], F32,
                                                     tag="sc", name="sc")
                                  for j in range(KG):
                                      kb = kg * KG + j
                                      nc.tensor.matmul(
                                          sct[:, j * QB:(j + 1) * QB],
                                          r(kT[hs, kb * KB:(kb + 1) * KB]),
                                          r(qT[hs, qb * QB:(qb + 1) * QB]),
                                          start=True, stop=True,
                                      )
                                  ut = u_pool.tile([128, KG * QB], MMDT, tag="u")
                                  nc.scalar.activation(
                                      out=ut, in_=sct,
                                      func=mybir.ActivationFunctionType.Exp,
                                      scale=1.0 / (HD ** 0.5),
                                  )
                                  for j in range(KG):
                                      kb = kg * KG + j
                                      nc.tensor.matmul(
                                          av[:, qb * QB:(qb + 1) * QB],
                                          r(va[:, 2 * kb + h, :]),
                                          r(ut[:, j * QB:(j + 1) * QB]),
                                          start=(kb == 0),
                                          stop=(kb == NKB - 1),
                                      )
                          # stage exp-weight sums into the stacked Uv tile
                          # and the denominator reciprocal straight from
                          # PSUM — both on DVE, overlapping later matmuls;
                          # then broadcast the reciprocal across partitions
                          # on the (idle) DMA queues
                          nc.vector.reciprocal(out=rr[(h, attn)],
                                               in_=av[HD:HD + 1, :])
                          nc.vector.tensor_copy(
                              Uv[attn][h * HD:(h + 1) * HD, :], av[0:HD, :])
                          row = 2 * (attn - 1) + h
                          nc.sync.dma_start(
                              out=d_rrs.ap()[row:row + 1, :],
                              in_=rr[(h, attn)])
                          nc.sync.dma_start(
                              out=rb[attn][h * HD:(h + 1) * HD, :],
                              in_=d_rrs.ap()[row:row + 1, :].to_broadcast(
                                  (HD, S)))
                          if attn == 2:
                              # per-head combine + bn_stats: head 0's chain
                              # overlaps head 1's attn2 block
                              nc.vector.scalar_tensor_tensor(
                                  out=t2[hs, :], in0=Uv[2][hs, :],
                                  scalar=neglamc[hs, :], in1=rb[2][hs, :],
                                  op0=mybir.AluOpType.mult,
                                  op1=mybir.AluOpType.mult,
                              )
                              nc.vector.tensor_add(oT[hs, :], n1[hs, :],
                                                   t2[hs, :])
                              for i in range(NST):
                                  nc.vector.bn_stats(
                                      out=bstats[hs, i, :],
                                      in_=oT[hs,
                                             i * BN_FMAX:(i + 1) * BN_FMAX])
                      if attn == 1:
                          # attn1's numerator scaling overlaps attn2's
                          # matmul/exp phase
                          nc.vector.tensor_mul(n1, Uv[1], rb[1])

              # ---- GroupNorm (whole [128, 2048] is one group) ----
              xn = o_pool.tile([CH, S], MMDT, tag="xn")
              with tc.tile_pool(name="stp", bufs=1, space="PSUM") as stp_pool:
                  mv = small.tile([CH, nc.vector.BN_AGGR_DIM], F32, tag="mv")
                  nc.vector.bn_aggr(out=mv, in_=bstats)
                  # per-partition [mean, E[x^2]] -> partition-sum via matmul
                  e2c = small.tile([CH, 1], F32, tag="e2c")
                  nc.vector.tensor_mul(e2c, mv[:, 0:1], mv[:, 0:1])
                  nc.vector.tensor_add(e2c, e2c, mv[:, 1:2])
                  st = stp_pool.tile([1, 2], F32, tag="st")
                  nc.tensor.matmul(st[0:1, 0:1], mv[:, 0:1], ones,
                                   start=True, stop=True)
                  nc.tensor.matmul(st[0:1, 1:2], e2c, ones,
                                   start=True, stop=True, skip_group_check=True)
                  mu_e2 = small.tile([1, 2], F32, tag="mu_e2")
                  nc.vector.tensor_scalar_mul(mu_e2, st[0:1, 0:2], 1.0 / CH)
                  # negvar = mu^2 - e2; std = sqrt(-negvar + eps)
                  negvar = small.tile([1, 1], F32, tag="negvar")
                  nc.vector.scalar_tensor_tensor(
                      out=negvar, in0=mu_e2[:, 0:1], scalar=mu_e2[:, 0:1],
                      in1=mu_e2[:, 1:2], op0=mybir.AluOpType.mult,
                      op1=mybir.AluOpType.subtract)
                  std = small.tile([1, 1], F32, tag="std")
                  nc.scalar.activation(out=std, in_=negvar,
                                       func=mybir.ActivationFunctionType.Sqrt,
                                       bias=eps_t, scale=-1.0)
                  # reciprocal lands directly in murstd (mu_e2 doubles as
                  # the [mu, rstd] pair; the e2 slot is dead after negvar)
                  murstd = mu_e2
                  nc.vector.reciprocal(out=murstd[:, 1:2], in_=std)
                  # broadcast [mu, rstd] across partitions via K=1 PE matmul
                  brp = stp_pool.tile([CH, 2], F32, tag="brp")
                  nc.tensor.matmul(brp, onesr, murstd, start=True, stop=True,
                                   skip_group_check=True)
                  a_t = small.tile([CH, 1], F32, tag="a_t")
                  nc.vector.tensor_mul(a_t, brp[:, 1:2], gnw)
                  # negb = a*mu - gnb;  xn = a*oT - negb
                  negb = small.tile([CH, 1], F32, tag="negb")
                  nc.vector.scalar_tensor_tensor(
                      out=negb, in0=brp[:, 0:1], scalar=a_t, in1=gnb,
                      op0=mybir.AluOpType.mult,
                      op1=mybir.AluOpType.subtract)
                  # xn in 4 token-chunks so the final projection can start
                  # on chunk 0 while the rest are still normalizing
                  for cqb in range(4):
                      cs = slice(cqb * QB, (cqb + 1) * QB)
                      nc.vector.tensor_scalar(out=xn[:, cs], in0=oT[:, cs],
                                              scalar1=a_t, scalar2=negb,
                                              op0=mybir.AluOpType.mult,
                                              op1=mybir.AluOpType.subtract)

              # ---- final projection partial: y = xn.T @ owT ----
              # 4 MMs fill a [128, 2048] (4-bank) PSUM tile -> 1 copy each
              with tc.tile_pool(name="fin", bufs=2, space="PSUM") as fin_pool, \
                   tc.tile_pool(name="ytp", bufs=2) as yt_pool:
                  half = NSB // 4
                  for hf in range(4):
                      ps = fin_pool.tile([SB, half * D], F32, tag="fin",
                                         name="fin")
                      yt = yt_pool.tile([SB, half, D], F32, tag="yt",
                                        name=f"yt{hf}")
                      for i in range(half):
                          sb = hf * half + i
                          nc.tensor.matmul(
                              ps[:, i * D:(i + 1) * D],
                              r(xn[:, sb * SB:(sb + 1) * SB]),
                              r(owT),
                              start=True, stop=True,
                          )
                      nc.vector.tensor_copy(yt, ps.rearrange(
                          "p (i d) -> p i d", i=half))
                      nc.sync.dma_start(
                          out=d_y.ap().rearrange(
                              "p (hf sb d) -> p hf sb d", hf=4, sb=half)[:, hf],
                          in_=yt)

    nc.compile()
    return nc


def _shard_inputs(inputs):
    import ml_dtypes
    mmnp = ml_dtypes.bfloat16 if USE_BF16 else np.float32
    x = np.ascontiguousarray(inputs["x"], np.float32)
    lam = (np.exp(inputs["lambda_q1"] * inputs["lambda_k1"])
           - np.exp(inputs["lambda_q2"] * inputs["lambda_k2"])
           + LAMBDA_INIT).astype(np.float32).reshape(H)
    in_maps = []
    for c in range(N_CORES):
        b, g = divmod(c, 4)
        ch = slice(CH * g, CH * (g + 1))
        # xp[p, c, s] = x[b, s, 128c+p]
        xp = np.ascontiguousarray(
            x[b].T.reshape(4, 128, S).transpose(1, 0, 2).reshape(128, 4 * S)
        ).astype(mmnp)
        wlist = []
        for W in (inputs["Q1_w"], inputs["K1_w"], inputs["Q2_w"],
                  inputs["K2_w"], inputs["V_w"]):
            wT = np.asarray(W)[ch].T  # [512, 128]
            wlist.append(np.ascontiguousarray(
                wT.reshape(4, 128, CH).transpose(1, 0, 2).reshape(128, 512)))
        owT = np.ascontiguousarray(np.asarray(inputs["out_w"])[:, ch].T)
        wp = np.concatenate(wlist + [owT], axis=1).astype(mmnp)
        neglam_packed = np.concatenate([
            np.full(HD, -lam[2 * g], np.float32),
            np.full(HD, -lam[2 * g + 1], np.float32)])
        cp = np.stack([
            np.asarray(inputs["K1_b"])[ch],
            np.asarray(inputs["K2_b"])[ch],
            np.asarray(inputs["gn_w"])[ch],
            np.asarray(inputs["gn_b"])[ch],
            np.full(CH, -lam[2 * g], np.float32),
            np.full(CH, -lam[2 * g + 1], np.float32),
            neglam_packed,
        ], axis=1).astype(np.float32)
        in_maps.append({"xp": xp, "wp": wp, "cp": np.ascontiguousarray(cp)})
    return in_maps


def kernel(**inputs):
    inputs = {k: np.asarray(v) for k, v in inputs.items()}
    if "nc" not in _CACHE:
        _CACHE["nc"] = build_program()
    nc = _CACHE["nc"]
    in_maps = _shard_inputs(inputs)
    res = run_bass_kernel_spmd(nc, in_maps, list(range(N_CORES)))
    out_b = np.asarray(inputs["out_b"], np.float32)
    y = np.zeros((B, S, D), np.float32)
    for c in range(N_CORES):
        b = c // 4
        yp = res.results[c]["y_part"].astype(np.float32)
        y[b] += yp.reshape(SB, NSB, D).transpose(1, 0, 2).reshape(S, D)
    y += out_b[None, None, :]
    return y



# revision 37
# speedup vs baseline: 2.1004x; 1.0732x over previous
"""MultiHeadDiffAttention TRN2 kernel.

Sharding: 8 cores = 2 batches x 4 head-pairs. Core c handles batch c//4 and
heads {2g, 2g+1} where g = c%4. The 2 heads = 128 channels = exactly one
GroupNorm group, so GroupNorm is core-local. The final projection is computed
as a partial sum over the core's 128 channels; the host adds the 4 partials
per batch plus the output bias.

Layout: "channel-major" [channels(partitions), sequence(free)] everywhere.
  - host pre-transposes/packs x and weights into per-partition-contiguous
    layouts so each big input is a single DMA
  - q/k projections land as qT/kT [128(2 heads x 64hd), 2048]
  - scores are computed transposed: S_T[k, q] (keys on partitions) so the
    exp'd scores feed the attn@V matmul directly as the moving operand
  - v is produced token-major [s, hd] with a ones-column appended, so the
    attn@V matmul also yields the softmax denominator (row 64 of PSUM out)
  - softmax skips max-subtraction: scores are bounded (|s|<2 for this data
    distribution, exp is exact in fp32)
  - exp-weight sums are staged into per-attn [128, 2048] Uv tiles (both
    heads stacked on partitions) so the diff-attn combine is 3 full-width
    DVE ops

Pipelining (engine overlap is what the runtime rewards):
  - score PSUM groups are [128, 1024] (2 banks) double-buffered next to the
    4-bank attn@V accumulator, so the PE never stalls on the exp drain
  - denominator reciprocals are taken straight from PSUM during attention;
    their across-partition broadcast runs on the DMA queues (SBUF -> DRAM
    scratch -> stride-0 broadcast back) while the PE keeps computing
  - attn1's numerator scaling and head 0's combine + bn_stats run under
    attn2's matmul/exp phase; only head 1's short chain remains in the tail
  - input x and the weights arrive via per-chunk DMAs so the first
    projection starts ~4 us in; yt staging is double-buffered so the
    final-projection copy/DMA pipeline never blocks the PE
"""

import os
import sys

sys.path.insert(0, "/opt/trn_rl_repo")

import numpy as np

import concourse.bacc as bacc
import concourse.bass as bass
import concourse.mybir as mybir
import concourse.tile as tile
from concourse.masks import make_identity
from concourse.bass_utils import run_bass_kernel_spmd

B, S, D = 2, 2048, 512
H = 8
G = 4
HD = D // H          # 64
CH = 2 * HD          # 128 channels per core (one GroupNorm group)
LAMBDA_INIT = 0.2
EPS = 1e-5
N_CORES = 8

QB = 512             # query block (matmul N)
NQB = S // QB        # 4
KB = 128             # key block (matmul M)
NKB = S // KB        # 16
KG = 2               # key blocks per exp group ([128, 1024] PSUM tile);
                     # small enough that sc_pool can double-buffer (2 banks
                     # x 2 bufs) next to the 4-bank av accumulator, so the
                     # PE never stalls waiting for the exp to drain PSUM
NKG = NKB // KG      # 8
SB = 128             # seq block for v / final matmul
NSB = S // SB        # 16

F32 = mybir.dt.float32
F32R = mybir.dt.float32r
BF16 = mybir.dt.bfloat16

USE_BF16 = os.environ.get("KERNEL_BF16", "0") == "1"
MMDT = BF16 if USE_BF16 else F32R
NWEIGHTS = 5

_CACHE = {}


def r(ap):
    """bitcast an fp32-typed AP to float32r (no-op for bf16 tiles)"""
    if USE_BF16:
        return ap
    return ap.bitcast(F32R)


def build_program(repeats=1):
    nc = bacc.Bacc("TRN2", target_bir_lowering=False, debug=False)

    mmdt_in = MMDT if USE_BF16 else F32

    # ---- external I/O (packed per-partition-contiguous host layouts) ----
    # xp[p, c, s] = x[b, s, 128c+p]
    d_xp = nc.declare_dram_parameter("xp", [128, 4 * S], mmdt_in, isOutput=False)
    # wp[p, w, c, m] = W_w[ch0+m, 128c+p]; w in (q1,k1,q2,k2,v); + owT at tail
    d_wp = nc.declare_dram_parameter("wp", [128, NWEIGHTS * 512 + D],
                                     mmdt_in, isOutput=False)
    # cp[p, :] = [k1b, k2b, gnw, gnb, neglam0, neglam1, neglam_packed]
    d_cp = nc.declare_dram_parameter("cp", [CH, 7], F32, isOutput=False)
    # yp[p, sb, d] = y_part[128*sb+p, d]
    d_y = nc.declare_dram_parameter("y_part", [SB, NSB * D], F32, isOutput=True)

    with tile.TileContext(nc) as tc:
      for _rep in range(repeats):
        with (
            tc.tile_pool(name="consts", bufs=1) as consts,
            tc.tile_pool(name="qk", bufs=1) as qk_pool,
            tc.tile_pool(name="vaug", bufs=1) as vaug_pool,
        ):
            # ---- constants / packed inputs ----
            ones = consts.tile([128, 1], F32, tag="ones")
            nc.vector.memset(ones, 1.0)
            onesr = consts.tile([1, 128], F32, tag="onesr")
            nc.vector.memset(onesr, 1.0)
            eps_t = consts.tile([1, 1], F32, tag="eps")
            nc.vector.memset(eps_t, EPS)
            cp = consts.tile([CH, 7], F32, tag="cp")
            nc.sync.dma_start(out=cp, in_=d_cp.ap())
            k1b, k2b = cp[:, 0:1], cp[:, 1:2]
            gnw, gnb = cp[:, 2:3], cp[:, 3:4]
            neglam = cp[:, 4:6]
            neglamc = cp[:, 6:7]

            wt = consts.tile([128, NWEIGHTS, 4, CH], MMDT, tag="wt")
            owT = consts.tile([CH, D], MMDT, tag="owT")
            wp_ap = d_wp.ap() if USE_BF16 else d_wp.ap().bitcast(F32R)
            WIDX = {"q1": 0, "k1": 1, "q2": 2, "k2": 3, "v": 4}
            wp_r = wp_ap[:, 0:NWEIGHTS * 512].rearrange(
                "p (w c m) -> p w c m", w=NWEIGHTS, c=4)
            # per-weight DMAs, v first (it is projected first); the x
            # chunks are interleaved below so wt[v]/xt[0] land first

            # ---- projections: qT/kT [128, 2048] channel-major ----
            qk = {}
            identf = consts.tile([SB, SB], F32, tag="identf")
            make_identity(nc, identf)
            ident = consts.tile([SB, SB], MMDT, tag="ident")
            nc.vector.tensor_copy(ident, identf)
            vT = qk_pool.tile([CH, S], MMDT, tag="vT")
            with (
                tc.tile_pool(name="xtp", bufs=1) as xt_pool,
                tc.tile_pool(name="pj", bufs=2, space="PSUM") as pj_pool,
            ):
                xt = xt_pool.tile([128, 4, S], MMDT, tag="xt")
                xp_src = (d_xp.ap() if USE_BF16
                          else d_xp.ap().bitcast(F32R)).rearrange(
                              "p (c s) -> p c s", c=4)
                # per-chunk DMAs interleaved with the weight DMAs so the
                # first projection starts as early as possible
                worder = ("v", "k1", "q1", "k2", "q2")
                nc.sync.dma_start(out=wt[:, WIDX["v"]], in_=wp_r[:, WIDX["v"]])
                for c in range(4):
                    nc.sync.dma_start(out=xt[:, c], in_=xp_src[:, c])
                    w = worder[c + 1]
                    nc.sync.dma_start(out=wt[:, WIDX[w]], in_=wp_r[:, WIDX[w]])
                nc.sync.dma_start(out=owT, in_=wp_ap[:, NWEIGHTS * 512:])

                # v first: its transposes feed va, needed by the AV matmuls
                for w, bias in (("v", None), ("k1", k1b), ("q1", None),
                                ("k2", k2b), ("q2", None)):
                    dst = vT if w == "v" else qk_pool.tile([CH, S], MMDT,
                                                           tag=w)
                    if w != "v":
                        qk[w] = dst
                    ps = pj_pool.tile([CH, 4 * QB], F32, tag="pj",
                                      name=f"pj_{w}")
                    # c outer: matmuls start as soon as x-chunk 0 lands
                    # instead of waiting for the whole 4 MB input
                    for c in range(4):
                        for qb in range(NQB):
                            nc.tensor.matmul(
                                ps[:, qb * QB:(qb + 1) * QB],
                                wt[:, WIDX[w], c, :],
                                xt[:, c, qb * QB:(qb + 1) * QB],
                                start=(c == 0),
                                stop=(c == 3),
                            )
                    if bias is not None:
                        nc.vector.tensor_scalar_add(dst, ps, bias)
                    else:
                        nc.vector.tensor_copy(dst, ps)

            # PE-transpose v per 128-block into token-major va [s, hd] with
            # a ones column; slot 2*sb+h
            with tc.tile_pool(name="pv", bufs=4, space="PSUM") as pv_pool:
                va = vaug_pool.tile([SB, 2 * NSB, HD + 1], MMDT, tag="va")
                nc.vector.tensor_copy(
                    va[:, :, HD:HD + 1],
                    ones.to_broadcast((SB, 2 * NSB, 1)))
                for sb in range(NSB):
                    pst = pv_pool.tile([SB, SB], MMDT, tag="pv", name="pv")
                    nc.tensor.transpose(
                        pst, vT[:, sb * SB:(sb + 1) * SB], ident)
                    nc.vector.tensor_copy(
                        va[:, 2 * sb:2 * sb + 2, 0:HD],
                        pst.rearrange("p (h m) -> p h m", h=2))

            # ---- attention-phase pools (reuse the x-tile region) ----
            with (
                tc.tile_pool(name="upool", bufs=4) as u_pool,
                tc.tile_pool(name="ubig", bufs=1) as ubig_pool,
                tc.tile_pool(name="opool", bufs=1) as o_pool,
                tc.tile_pool(name="small", bufs=1) as small,
            ):
              # ---- attention ----
              oT = o_pool.tile([CH, S], F32, tag="oT")
              # Uv[attn]: [128, 2048] exp-weight sums, both heads stacked on
              # partitions; rr[(h, attn)]: per-token reciprocal denominators;
              # rb[attn]: rr DMA-broadcast across partitions (runs on idle
              # DMA queues during attention)
              Uv = {a: ubig_pool.tile([CH, S], F32, tag=f"Uv{a}",
                                      name=f"Uv{a}")
                    for a in (1, 2)}
              rr = {(h, a): small.tile([1, S], F32, tag=f"rr{h}{a}",
                                       name=f"rr{h}{a}")
                    for h in (0, 1) for a in (1, 2)}
              rb = {a: ubig_pool.tile([CH, S], F32, tag=f"rb{a}",
                                      name=f"rb{a}")
                    for a in (1, 2)}
              d_rrs = nc.dram_tensor(f"rr_scratch_{_rep}", (4, S), F32,
                                     kind="Internal")
              n1 = small.tile([CH, S], F32, tag="n1")
              t2 = small.tile([CH, S], F32, tag="t2")
              BN_FMAX = nc.vector.BN_STATS_FMAX
              NST = S // BN_FMAX
              bstats = small.tile([CH, NST, nc.vector.BN_STATS_DIM], F32,
                                  tag="bstats")

              with (
                  tc.tile_pool(name="sc", bufs=2, space="PSUM") as sc_pool,
                  tc.tile_pool(name="av", bufs=1, space="PSUM") as av_pool,
              ):
                  for attn in (1, 2):
                      qT, kT = qk[f"q{attn}"], qk[f"k{attn}"]
                      for h in (0, 1):
                          hs = slice(h * HD, (h + 1) * HD)
                          av = av_pool.tile([HD + 1, NQB * QB], F32,
                                            tag="av", name="av")

                          # software-pipelined: AV(i) is emitted after
                          # S(i+1)/exp(i+1), so the in-order PE queue works
                          # on the next score group instead of stalling on
                          # exp(i)'s result
                          def emit_scores(qb, kg):
                              sct = sc_pool.tile([128, KG * QB], F32,
                                                 tag="sc", name="sc")
                              for j in range(KG):
                                  kb = kg * KG + j
                                  nc.tensor.matmul(
                                      sct[:, j * QB:(j + 1) * QB],
                                      r(kT[hs, kb * KB:(kb + 1) * KB]),
                                      r(qT[hs, qb * QB:(qb + 1) * QB]),
                                      start=True, stop=True,
                                  )
                              ut = u_pool.tile([128, KG * QB], MMDT, tag="u")
                              nc.scalar.activation(
                                  out=ut, in_=sct,
                                  func=mybir.ActivationFunctionType.Exp,
                                  scale=1.0 / (HD ** 0.5),
                              )
                              return ut

                          def emit_av(qb, kg, ut):
                              for j in range(KG):
                                  kb = kg * KG + j
                                  nc.tensor.matmul(
                                      av[:, qb * QB:(qb + 1) * QB],
                                      r(va[:, 2 * kb + h, :]),
                                      r(ut[:, j * QB:(j + 1) * QB]),
                                      start=(kb == 0),
                                      stop=(kb == NKB - 1),
                                  )

                          prev = None
                          for qb in range(NQB):
                              for kg in range(NKG):
                                  ut = emit_scores(qb, kg)
                                  if prev is not None:
                                      emit_av(*prev)
                                  prev = (qb, kg, ut)
                          emit_av(*prev)
                          # stage exp-weight sums into the stacked Uv tile
                          # and the denominator reciprocal straight from
                          # PSUM — both on DVE, overlapping later matmuls;
                          # then broadcast the reciprocal across partitions
                          # on the (idle) DMA queues
                          nc.vector.reciprocal(out=rr[(h, attn)],
                                               in_=av[HD:HD + 1, :])
                          nc.vector.tensor_copy(
                              Uv[attn][h * HD:(h + 1) * HD, :], av[0:HD, :])
                          row = 2 * (attn - 1) + h
                          nc.sync.dma_start(
                              out=d_rrs.ap()[row:row + 1, :],
                              in_=rr[(h, attn)])
                          nc.sync.dma_start(
                              out=rb[attn][h * HD:(h + 1) * HD, :],
                              in_=d_rrs.ap()[row:row + 1, :].to_broadcast(
                                  (HD, S)))
                          if attn == 2:
                              # per-head combine + bn_stats: head 0's chain
                              # overlaps head 1's attn2 block
                              nc.vector.scalar_tensor_tensor(
                                  out=t2[hs, :], in0=Uv[2][hs, :],
                                  scalar=neglamc[hs, :], in1=rb[2][hs, :],
                                  op0=mybir.AluOpType.mult,
                                  op1=mybir.AluOpType.mult,
                              )
                              nc.vector.tensor_add(oT[hs, :], n1[hs, :],
                                                   t2[hs, :])
                              for i in range(NST):
                                  nc.vector.bn_stats(
                                      out=bstats[hs, i, :],
                                      in_=oT[hs,
                                             i * BN_FMAX:(i + 1) * BN_FMAX])
                      if attn == 1:
                          # attn1's numerator scaling overlaps attn2's
                          # matmul/exp phase
                          nc.vector.tensor_mul(n1, Uv[1], rb[1])

              # ---- GroupNorm (whole [128, 2048] is one group) ----
              xn = o_pool.tile([CH, S], MMDT, tag="xn")
              with tc.tile_pool(name="stp", bufs=1, space="PSUM") as stp_pool:
                  mv = small.tile([CH, nc.vector.BN_AGGR_DIM], F32, tag="mv")
                  nc.vector.bn_aggr(out=mv, in_=bstats)
                  # per-partition [mean, E[x^2]] -> partition-sum via matmul
                  e2c = small.tile([CH, 1], F32, tag="e2c")
                  nc.vector.tensor_mul(e2c, mv[:, 0:1], mv[:, 0:1])
                  nc.vector.tensor_add(e2c, e2c, mv[:, 1:2])
                  st = stp_pool.tile([1, 2], F32, tag="st")
                  nc.tensor.matmul(st[0:1, 0:1], mv[:, 0:1], ones,
                                   start=True, stop=True)
                  nc.tensor.matmul(st[0:1, 1:2], e2c, ones,
                                   start=True, stop=True, skip_group_check=True)
                  mu_e2 = small.tile([1, 2], F32, tag="mu_e2")
                  nc.vector.tensor_scalar_mul(mu_e2, st[0:1, 0:2], 1.0 / CH)
                  # negvar = mu^2 - e2; std = sqrt(-negvar + eps)
                  negvar = small.tile([1, 1], F32, tag="negvar")
                  nc.vector.scalar_tensor_tensor(
                      out=negvar, in0=mu_e2[:, 0:1], scalar=mu_e2[:, 0:1],
                      in1=mu_e2[:, 1:2], op0=mybir.AluOpType.mult,
                      op1=mybir.AluOpType.subtract)
                  std = small.tile([1, 1], F32, tag="std")
                  nc.scalar.activation(out=std, in_=negvar,
                                       func=mybir.ActivationFunctionType.Sqrt,
                                       bias=eps_t, scale=-1.0)
                  # reciprocal lands directly in murstd (mu_e2 doubles as
                  # the [mu, rstd] pair; the e2 slot is dead after negvar)
                  murstd = mu_e2
                  nc.vector.reciprocal(out=murstd[:, 1:2], in_=std)
                  # broadcast [mu, rstd] across partitions via K=1 PE matmul
                  brp = stp_pool.tile([CH, 2], F32, tag="brp")
                  nc.tensor.matmul(brp, onesr, murstd, start=True, stop=True,
                                   skip_group_check=True)
                  a_t = small.tile([CH, 1], F32, tag="a_t")
                  nc.vector.tensor_mul(a_t, brp[:, 1:2], gnw)
                  # negb = a*mu - gnb;  xn = a*oT - negb
                  negb = small.tile([CH, 1], F32, tag="negb")
                  nc.vector.scalar_tensor_tensor(
                      out=negb, in0=brp[:, 0:1], scalar=a_t, in1=gnb,
                      op0=mybir.AluOpType.mult,
                      op1=mybir.AluOpType.subtract)
                  # xn in 4 token-chunks so the final projection can start
                  # on chunk 0 while the rest are still normalizing
                  for cqb in range(4):
                      cs = slice(cqb * QB, (cqb + 1) * QB)
                      nc.vector.tensor_scalar(out=xn[:, cs], in0=oT[:, cs],
                                              scalar1=a_t, scalar2=negb,
                                              op0=mybir.AluOpType.mult,
                                              op1=mybir.AluOpType.subtract)

              # ---- final projection partial: y = xn.T @ owT ----
              # 4 MMs fill a [128, 2048] (4-bank) PSUM tile -> 1 copy each
              with tc.tile_pool(name="fin", bufs=2, space="PSUM") as fin_pool, \
                   tc.tile_pool(name="ytp", bufs=2) as yt_pool:
                  half = NSB // 4
                  for hf in range(4):
                      ps = fin_pool.tile([SB, half * D], F32, tag="fin",
                                         name="fin")
                      yt = yt_pool.tile([SB, half, D], F32, tag="yt",
                                        name=f"yt{hf}")
                      for i in range(half):
                          sb = hf * half + i
                          nc.tensor.matmul(
                              ps[:, i * D:(i + 1) * D],
                              r(xn[:, sb * SB:(sb + 1) * SB]),
                              r(owT),
                              start=True, stop=True,
                          )
                      nc.vector.tensor_copy(yt, ps.rearrange(
                          "p (i d) -> p i d", i=half))
                      nc.sync.dma_start(
                          out=d_y.ap().rearrange(
                              "p (hf sb d) -> p hf sb d", hf=4, sb=half)[:, hf],
                          in_=yt)

    nc.compile()
    return nc


def _shard_inputs(inputs):
    import ml_dtypes
    mmnp = ml_dtypes.bfloat16 if USE_BF16 else np.float32
    x = np.ascontiguousarray(inputs["x"], np.float32)
    lam = (np.exp(inputs["lambda_q1"] * inputs["lambda_k1"])
           - np.exp(inputs["lambda_q2"] * inputs["lambda_k2"])
           + LAMBDA_INIT).astype(np.float32).reshape(H)
    in_maps = []
    for c in range(N_CORES):
        b, g = divmod(c, 4)
        ch = slice(CH * g, CH * (g + 1))
        # xp[p, c, s] = x[b, s, 128c+p]
        xp = np.ascontiguousarray(
            x[b].T.reshape(4, 128, S).transpose(1, 0, 2).reshape(128, 4 * S)
        ).astype(mmnp)
        wlist = []
        for W in (inputs["Q1_w"], inputs["K1_w"], inputs["Q2_w"],
                  inputs["K2_w"], inputs["V_w"]):
            wT = np.asarray(W)[ch].T  # [512, 128]
            wlist.append(np.ascontiguousarray(
                wT.reshape(4, 128, CH).transpose(1, 0, 2).reshape(128, 512)))
        owT = np.ascontiguousarray(np.asarray(inputs["out_w"])[:, ch].T)
        wp = np.concatenate(wlist + [owT], axis=1).astype(mmnp)
        neglam_packed = np.concatenate([
            np.full(HD, -lam[2 * g], np.float32),
            np.full(HD, -lam[2 * g + 1], np.float32)])
        cp = np.stack([
            np.asarray(inputs["K1_b"])[ch],
            np.asarray(inputs["K2_b"])[ch],
            np.asarray(inputs["gn_w"])[ch],
            np.asarray(inputs["gn_b"])[ch],
            np.full(CH, -lam[2 * g], np.float32),
            np.full(CH, -lam[2 * g + 1], np.float32),
            neglam_packed,
        ], axis=1).astype(np.float32)
        in_maps.append({"xp": xp, "wp": wp, "cp": np.ascontiguousarray(cp)})
    return in_maps


def kernel(**inputs):
    inputs = {k: np.asarray(v) for k, v in inputs.items()}
    if "nc" not in _CACHE:
        _CACHE["nc"] = build_program()
    nc = _CACHE["nc"]
    in_maps = _shard_inputs(inputs)
    res = run_bass_kernel_spmd(nc, in_maps, list(range(N_CORES)))
    out_b = np.asarray(inputs["out_b"], np.float32)
    y = np.zeros((B, S, D), np.float32)
    for c in range(N_CORES):
        b = c // 4
        yp = res.results[c]["y_part"].astype(np.float32)
        y[b] += yp.reshape(SB, NSB, D).transpose(1, 0, 2).reshape(S, D)
    y += out_b[None, None, :]
    return y

